# revision 1
# baseline (speedup 1.0000x reference)
"""DynaLoRALinear Trainium2 kernel.

Data-parallel over batch B across 8 NeuronCores (one sample per core).
Per core:
  - router:  logits = pooled @ (W_r @ gating_W).T  computed as a sharded
    partial (each core contracts over a 512-wide slice of D) + AllReduce.
  - gate weights from expert_scores ranks + module_prob>0.5 branch select.
  - base:    out = x_b @ W_base.T + b_base   (tf32 matmuls, fp32 PSUM accum)
  - lora:    t = x_b @ A_cat.T (fused into chunk-0 k-loop), then
             out += t @ (B_cat * gate).T
Matmuls use float32r (tf32) operands pre-rounded on host: 1 cyc/row on PE
(4x faster than fp32) at ~3e-4 scale-relative absmax error.
"""

import sys
import types

import numpy as np

B, L, D, E, R, NMOD = 8, 2048, 4096, 4, 8, 7
N_CORES = 8
DSH = D // N_CORES  # 512: per-core slice of D for the router shard
ER = E * R          # 32
O_C = 1024          # W_base column chunk cached in SBUF
N_CHUNK = D // O_C  # 4
KT = D // 128       # 32 k-tiles
XB = 8              # k-tiles batched per x DMA
MT = L // 128       # 16 m-tiles


def _round_tf32(a) -> np.ndarray:
    """Round-to-nearest-even fp32 -> tf32 (10-bit mantissa), keep fp32 bits."""
    a = np.ascontiguousarray(a, dtype=np.float32)
    u = a.view(np.uint32).astype(np.uint64)
    u = (u + 0xFFF + ((u >> 13) & 1)) & 0xFFFFE000
    return np.ascontiguousarray(u.astype(np.uint32)).view(np.float32)


def _install_profile_hook():
    """Make bass_utils' trace path importable (no-op if already present)."""
    try:
        import antenv.axon_hooks  # noqa: F401
        return
    except ImportError:
        pass
    try:
        import antenv
    except ImportError:
        return
    mod = types.ModuleType("antenv.axon_hooks")
    mod._hook = None
    mod.set_axon_ntff_profile_hook = lambda h: setattr(mod, "_hook", h)
    mod.get_axon_ntff_profile_hook = lambda: mod._hook
    sys.modules["antenv.axon_hooks"] = mod
    antenv.axon_hooks = mod
    try:
        from trn_agent_boot.trn_boot import _ntff_profile_via_ctypes
        hook = _ntff_profile_via_ctypes("/opt/axon/libaxon_pjrt.so")
        if hook is not None:
            mod.set_axon_ntff_profile_hook(hook)
    except Exception:
        pass


_PROGRAM_CACHE = {}


def _build_program(k: int, module_idx: int, has_bias: bool):
    import concourse.mybir as mybir
    import concourse.tile as tile
    from concourse import bacc
    from concourse.masks import make_identity

    f32 = mybir.dt.float32
    f32r = mybir.dt.float32r
    alu = mybir.AluOpType
    act_fn = mybir.ActivationFunctionType

    k_lo = max(1, k // 2)

    nc = bacc.Bacc("TRN2", target_bir_lowering=False, debug=False,
                   num_devices=N_CORES)

    # --- DRAM I/O -------------------------------------------------------
    xT = nc.dram_tensor("xT", [D, L], f32r, kind="ExternalInput")
    WbT = nc.dram_tensor("WbT", [D, D], f32r, kind="ExternalInput")
    gw = nc.dram_tensor("gw", [D, DSH], f32r, kind="ExternalInput")
    WrT = nc.dram_tensor("WrT", [D, NMOD], f32r, kind="ExternalInput")
    pooledT = nc.dram_tensor("pooledT", [DSH, B], f32, kind="ExternalInput")
    scores_f = nc.dram_tensor("scores_f", [1, E * B], f32,
                              kind="ExternalInput")
    A_rhs = nc.dram_tensor("A_rhs", [D, ER], f32r, kind="ExternalInput")
    B_cat = nc.dram_tensor("B_cat", [ER, D], f32, kind="ExternalInput")
    b_row = nc.dram_tensor("b_row", [1, D], f32, kind="ExternalInput")
    msel = nc.dram_tensor("msel", [ER, E * B], f32, kind="ExternalInput")
    out = nc.dram_tensor("out", [L, D], f32, kind="ExternalOutput")

    with tile.TileContext(nc) as tc:
        with (
            tc.tile_pool(name="const", bufs=1) as const_pool,
            tc.tile_pool(name="gatep", bufs=1) as gate_pool,
            tc.tile_pool(name="rsb", bufs=1) as rsb,
            tc.tile_pool(name="wpool",
                         bufs=2 * KT + (4 if has_bias else 4)) as wpool,
            tc.tile_pool(name="xpool", bufs=10) as xpool,
            tc.tile_pool(name="apool", bufs=1) as apool,
            tc.tile_pool(name="tpool", bufs=4) as tpool,
            tc.tile_pool(name="bpool", bufs=2) as bpool,
            tc.tile_pool(name="biasp", bufs=D // 512) as biasp,
            tc.tile_pool(name="epool", bufs=4) as epool,
        ):
            ident = const_pool.tile([128, 128], f32)
            make_identity(nc, ident)
            gate32 = gate_pool.tile([ER, 1], f32)

            bias_all = []
            if has_bias:
                for hh in range(D // 512):
                    bias_bc = biasp.tile([128, 512], f32, tag="biasbc",
                                         name=f"biasbc_{hh}")
                    nc.sync.dma_start(
                        bias_bc[0:1, :],
                        b_row[:, hh * 512:(hh + 1) * 512])
                    nc.gpsimd.partition_broadcast(bias_bc[:],
                                                  bias_bc[0:1, :])
                    bias_all.append(bias_bc)

            # ====== router part A: matmuls + AllReduce trigger =========
            # (everything that depends on the AllReduce result is emitted
            # AFTER chunk 0, so the collective never blocks the in-order
            # engine queues ahead of chunk-0 work.)
            wtiles0 = []
            with (
                tc.tile_pool(name="rgw", bufs=6) as rgw,
                tc.tile_pool(name="rps", bufs=1, space="PSUM") as rps,
                tc.tile_pool(name="rdram", bufs=1, space="DRAM") as rdram,
            ):
                wr_sb = rsb.tile([128, KT, NMOD], f32r)
                nc.sync.dma_start(
                    wr_sb[:], WrT[:].rearrange("(a p) m -> p a m", p=128))
                wc_ps = rps.tile([NMOD, DSH], f32)
                for kt in range(KT):
                    gwt = rgw.tile([128, DSH], f32r, tag="gwt",
                                   name=f"gwt_{kt}")
                    nc.sync.dma_start(gwt[:], gw[kt * 128:(kt + 1) * 128, :])
                    nc.tensor.matmul(wc_ps[:], wr_sb[:, kt, :], gwt[:],
                                     start=(kt == 0), stop=(kt == KT - 1))
                # small inputs, then A, then W chunk 0 stream
                pt_sb = rsb.tile([128, 4, B], f32)
                nc.sync.dma_start(
                    pt_sb[:],
                    pooledT[:].rearrange("(a p) m -> p a m", p=128))
                msel_sb = rsb.tile([ER, E * B], f32)
                nc.sync.dma_start(msel_sb[:], msel[:])
                sc = rsb.tile([1, E * B], f32)
                nc.sync.dma_start(sc[:], scores_f[:])
                a_sb = apool.tile([128, KT, ER], f32r)
                nc.sync.dma_start(
                    a_sb[:], A_rhs[:].rearrange("(a p) m -> p a m", p=128))
                for kt in range(KT):
                    wt = wpool.tile([128, 512], f32r, tag="w",
                                    name=f"w_0_{kt}")
                    nc.sync.dma_start(wt[:],
                                      WbT[kt * 128:(kt + 1) * 128, 0:512])
                    wtiles0.append(wt)

                wc_sb = rsb.tile([NMOD, DSH], f32)
                nc.vector.tensor_copy(wc_sb[:], wc_ps[:])
                wct = rsb.tile([128, 4 * NMOD], f32)
                for j in range(4):
                    tp = rps.tile([128, NMOD], f32, tag="tp", name=f"tp_{j}")
                    nc.tensor.transpose(
                        tp[:], wc_sb[:, j * 128:(j + 1) * 128],
                        ident[0:NMOD, 0:NMOD])
                    nc.vector.tensor_copy(
                        wct[:, j * NMOD:(j + 1) * NMOD], tp[:])

                lg_ps = rps.tile([NMOD, B], f32)
                for j in range(4):
                    nc.tensor.matmul(lg_ps[:],
                                     wct[:, j * NMOD:(j + 1) * NMOD],
                                     pt_sb[:, j, :],
                                     start=(j == 0), stop=(j == 3))
                lp_sb = rsb.tile([NMOD, B], f32)
                nc.vector.tensor_copy(lp_sb[:], lg_ps[:])

                cc_in = rdram.tile([NMOD, B], f32)
                cc_out = rdram.tile([NMOD, B], f32)
                nc.gpsimd.dma_start(cc_in[:], lp_sb[:])
                nc.gpsimd.collective_compute(
                    "AllReduce", alu.add,
                    replica_groups=[list(range(N_CORES))],
                    ins=[cc_in.opt()], outs=[cc_out.opt()])
                lg_sb = rsb.tile([NMOD, B], f32)
                nc.gpsimd.dma_start(lg_sb[:], cc_out[:])

                # collective-independent: expert ranks from scores
                rank = rsb.tile([1, E * B], f32)
                nc.vector.memset(rank[:], 0.0)
                tmp = rsb.tile([1, B], f32)
                for e in range(E):
                    re = rank[:, e * B:(e + 1) * B]
                    se = sc[:, e * B:(e + 1) * B]
                    for e2 in range(E):
                        if e2 == e:
                            continue
                        s2 = sc[:, e2 * B:(e2 + 1) * B]
                        nc.vector.tensor_tensor(tmp[:], s2, se, op=alu.is_gt)
                        nc.vector.tensor_add(re, re, tmp[:])
                        if e2 < e:
                            nc.vector.tensor_tensor(tmp[:], s2, se,
                                                    op=alu.is_equal)
                            nc.vector.tensor_add(re, re, tmp[:])
                w_hi = rsb.tile([1, E * B], f32)
                nc.vector.tensor_scalar(w_hi[:], rank[:], float(k),
                                        1.0 / float(k),
                                        op0=alu.is_lt, op1=alu.mult)
                w_lo = rsb.tile([1, E * B], f32)
                nc.vector.tensor_scalar(w_lo[:], rank[:], float(k_lo),
                                        1.0 / float(k_lo),
                                        op0=alu.is_lt, op1=alu.mult)
                diff = rsb.tile([1, E * B], f32)
                nc.vector.tensor_sub(diff[:], w_hi[:], w_lo[:])

            # ====== router part B (emitted after chunk 0 below) ========
            def emit_part_b(mps):
                ltp = mps.tile([B, NMOD], f32, tag="ps", name="ltp")
                nc.tensor.transpose(ltp[:], lg_sb[:], ident[0:NMOD, 0:NMOD])
                lt = rsb.tile([B, NMOD], f32)
                nc.vector.tensor_copy(lt[:], ltp[:])
                mx = rsb.tile([B, 1], f32)
                nc.vector.tensor_reduce(out=mx[:], in_=lt[:], op=alu.max,
                                        axis=mybir.AxisListType.X)
                mxn = rsb.tile([B, 1], f32)
                nc.vector.tensor_scalar_mul(mxn[:], mx[:], -1.0)
                ex = rsb.tile([B, NMOD], f32)
                nc.scalar.activation(ex[:], lt[:], act_fn.Exp, bias=mxn[:])
                sm = rsb.tile([B, 1], f32)
                nc.vector.tensor_reduce(out=sm[:], in_=ex[:], op=alu.add,
                                        axis=mybir.AxisListType.X)
                rs = rsb.tile([B, 1], f32)
                nc.vector.reciprocal(rs[:], sm[:])
                p0 = rsb.tile([B, 1], f32)
                nc.vector.tensor_mul(
                    p0[:], ex[:, module_idx:module_idx + 1], rs[:])
                hi = rsb.tile([B, 1], f32)
                nc.vector.tensor_single_scalar(hi[:], p0[:], 0.5, alu.is_gt)
                hp = mps.tile([1, B], f32, tag="ps", name="hp")
                nc.tensor.transpose(hp[:], hi[:], ident[0:B, 0:B])
                hi_row = rsb.tile([1, B], f32)
                nc.vector.tensor_copy(hi_row[:], hp[:])
                gate = rsb.tile([1, E * B], f32)
                for e in range(E):
                    nc.vector.tensor_mul(gate[:, e * B:(e + 1) * B],
                                         diff[:, e * B:(e + 1) * B],
                                         hi_row[:])
                nc.vector.tensor_add(gate[:], gate[:], w_lo[:])
                gateb = rsb.tile([ER, E * B], f32)
                nc.gpsimd.partition_broadcast(gateb[:], gate[:])
                g32m = rsb.tile([ER, E * B], f32)
                nc.vector.tensor_tensor(g32m[:], gateb[:], msel_sb[:],
                                        op=alu.mult)
                nc.vector.tensor_reduce(out=gate32[:], in_=g32m[:],
                                        op=alu.add,
                                        axis=mybir.AxisListType.X)

            # ============== main: base + lora ==========================
            with (
                tc.tile_pool(name="mps", bufs=8, space="PSUM") as mps,
            ):
                CHUNKS = [(0, 512), (512, 1024), (1536, 1024),
                          (2560, 1024), (3584, 512)]
                tT_tiles = [None] * (MT // 4)
                for c, (col0, width) in enumerate(CHUNKS):
                    nh = width // 512
                    GS = 4
                    NG = MT // GS
                    if c == 0:
                        wtiles = [[wtiles0[kt]] for kt in range(KT)]
                    else:
                        wtiles = []
                        for kt in range(KT):
                            row = []
                            for h in range(nh):
                                wt = wpool.tile([128, 512], f32r, tag="w",
                                                name=f"w_{c}_{kt}_{h}")
                                nc.sync.dma_start(
                                    wt[:],
                                    WbT[kt * 128:(kt + 1) * 128,
                                        col0 + h * 512:col0 + (h + 1) * 512])
                                row.append(wt)
                            wtiles.append(row)
                    # B chunk halves scaled by this core's gate.
                    # Chunk 0's scale must wait for part B (emitted in the
                    # c==0 tail below), so only stage its DMA here.
                    bh = []
                    bstg0 = []
                    for h in range(nh):
                        b_stg = bpool.tile([ER, 512], f32, tag="bstg",
                                           name=f"bstg_{c}_{h}")
                        nc.sync.dma_start(
                            b_stg[:],
                            B_cat[:, col0 + h * 512:col0 + (h + 1) * 512])
                        if c == 0:
                            bstg0.append(b_stg)
                            bh.append(None)
                            continue
                        b_scl = bpool.tile([ER, 512], f32r, tag="bscl",
                                           name=f"bscl_{c}_{h}")
                        nc.vector.tensor_scalar_mul(b_scl[:], b_stg[:],
                                                    gate32[:, 0:1])
                        bh.append(b_scl)

                    for mg in range(NG):
                        pss = []
                        for mi in range(GS):
                            row = []
                            for h in range(nh):
                                ps = mps.tile([128, 512], f32, tag="ps",
                                              name=f"ps_{c}_{mg}_{mi}_{h}")
                                row.append(ps)
                            pss.append(row)
                        if c == 0:
                            ps_t = mps.tile([ER, 512], f32, tag="ps",
                                            name=f"pst_{mg}")
                        for kt in range(KT):
                            xs = xpool.tile([128, GS * 128], f32r, tag="x",
                                            name=f"x_{c}_{mg}_{kt}")
                            nc.sync.dma_start(
                                xs[:],
                                xT[kt * 128:(kt + 1) * 128,
                                   mg * GS * 128:(mg + 1) * GS * 128])
                            if c == 0:
                                nc.tensor.matmul(
                                    ps_t[:], a_sb[:, kt, :], xs[:],
                                    start=(kt == 0), stop=(kt == KT - 1))
                            for mi in range(GS):
                                xsl = xs[:, mi * 128:(mi + 1) * 128]
                                for h in range(nh):
                                    nc.tensor.matmul(
                                        pss[mi][h][:], xsl, wtiles[kt][h][:],
                                        start=(kt == 0), stop=False)
                        if c == 0:
                            tT = tpool.tile([ER, 512], f32r, tag="tT",
                                            name=f"tT_{mg}")
                            nc.vector.tensor_copy(tT[:], ps_t[:])
                            tT_tiles[mg] = tT
                        for mi in range(GS):
                            m = mg * GS + mi
                            if c != 0:
                                tsl = tT_tiles[m // 4][:, (m % 4) * 128:
                                                       (m % 4) * 128 + 128]
                                for h in range(nh):
                                    nc.tensor.matmul(
                                        pss[mi][h][:], tsl, bh[h][:],
                                        start=False, stop=True)
                            for h in range(nh):
                                ev = epool.tile([128, 512], f32, tag="ev",
                                                name=f"ev_{c}_{m}_{h}")
                                if has_bias:
                                    nc.vector.tensor_add(
                                        ev[:], pss[mi][h][:],
                                        bias_all[(col0 // 512) + h][:])
                                elif h == 1:
                                    # spread psum eviction across ACT too:
                                    # frees bank slots ~2x faster at group
                                    # boundaries (all 8 banks per group)
                                    nc.scalar.activation(
                                        ev[:], pss[mi][h][:], act_fn.Copy)
                                else:
                                    nc.vector.tensor_copy(ev[:],
                                                          pss[mi][h][:])
                                nc.sync.dma_start(
                                    out[m * 128:(m + 1) * 128,
                                        col0 + h * 512:col0 + (h + 1) * 512],
                                    ev[:])
                    if c == 0:
                        # gate computation (needs the AllReduce result,
                        # which has landed by now on every core)
                        emit_part_b(mps)
                        b_scl0 = bpool.tile([ER, 512], f32r, tag="bscl",
                                            name="bscl_0_0")
                        nc.vector.tensor_scalar_mul(b_scl0[:], bstg0[0][:],
                                                    gate32[:, 0:1])
                        bh[0] = b_scl0
                        # deferred chunk-0 lora, accumulated via SWDGE
                        for m in range(MT):
                            tsl = tT_tiles[m // 4][:, (m % 4) * 128:
                                                   (m % 4) * 128 + 128]
                            lp = mps.tile([128, 512], f32, tag="ps",
                                          name=f"lp_{m}")
                            nc.tensor.matmul(lp[:], tsl, bh[0][:],
                                             start=True, stop=True)
                            lev = epool.tile([128, 512], f32, tag="ev",
                                             name=f"lev_{m}")
                            nc.vector.tensor_copy(lev[:], lp[:])
                            nc.gpsimd.dma_start(
                                out[m * 128:(m + 1) * 128, 0:512], lev[:],
                                accum_op=alu.add)

    nc.compile()
    return nc


def kernel(**inputs) -> np.ndarray:
    _install_profile_hook()

    x = np.asarray(inputs["x"], dtype=np.float32)
    expert_scores = np.asarray(inputs["expert_scores"], dtype=np.float32)
    W_base = np.asarray(inputs["W_base"], dtype=np.float32)
    b_base = np.asarray(inputs["b_base"], dtype=np.float32)
    gating_W = np.asarray(inputs["gating_W"], dtype=np.float32)
    W_r = np.asarray(inputs["W_r"], dtype=np.float32)
    lora_A = np.asarray(inputs["lora_A"], dtype=np.float32)
    lora_B = np.asarray(inputs["lora_B"], dtype=np.float32)
    module_idx = int(np.asarray(inputs["module_idx"]))
    k = int(np.asarray(inputs["k"]))

    has_bias = bool(np.any(b_base != 0.0))
    key = (k, module_idx, has_bias)
    if key not in _PROGRAM_CACHE:
        _PROGRAM_CACHE[key] = _build_program(k, module_idx, has_bias)
    nc = _PROGRAM_CACHE[key]

    # --- host-side layout prep (transposes/slices/rounding only) --------
    WbT_np = _round_tf32(W_base.T)                       # [D, D]
    WrT_np = _round_tf32(W_r.T)                          # [D, NMOD]
    A_np = _round_tf32(lora_A.reshape(ER, D).T)          # [D, ER]
    B_np = np.ascontiguousarray(
        lora_B.transpose(0, 2, 1).reshape(ER, D))        # [ER, D] fp32
    scores_f_np = np.ascontiguousarray(
        expert_scores.T.reshape(1, E * B))               # [1, E*B]
    b_row_np = b_base.reshape(1, D)
    pooled = x[:, -1, :]                                 # [B, D]

    in_maps = []
    for c in range(N_CORES):
        msel_np = np.zeros((ER, E, B), dtype=np.float32)
        for p in range(ER):
            msel_np[p, p // R, c] = 1.0
        msel_np = msel_np.reshape(ER, E * B)
        in_maps.append({
            "xT": _round_tf32(x[c].T),
            "WbT": WbT_np,
            "gw": _round_tf32(gating_W[:, c * DSH:(c + 1) * DSH]),
            "WrT": WrT_np,
            "pooledT": np.ascontiguousarray(
                pooled[:, c * DSH:(c + 1) * DSH].T),
            "scores_f": scores_f_np,
            "A_rhs": A_np,
            "B_cat": B_np,
            "b_row": b_row_np,
            "msel": msel_np,
        })

    from concourse.bass_utils import run_bass_kernel_spmd
    res = run_bass_kernel_spmd(nc, in_maps, core_ids=list(range(N_CORES)))
    return np.stack([res.results[c]["out"] for c in range(N_CORES)], axis=0)


if __name__ == "__main__":
    rng = np.random.default_rng(0)
    demo = {
        "x": (rng.standard_normal((B, L, D)) * 0.02).astype(np.float32),
        "expert_scores": rng.random((B, E), dtype=np.float32),
        "W_base": (rng.standard_normal((D, D)) * 0.02).astype(np.float32),
        "b_base": np.zeros(D, np.float32),
        "gating_W": (rng.standard_normal((D, D)) * 0.02).astype(np.float32),
        "W_r": (rng.standard_normal((NMOD, D)) * 0.02).astype(np.float32),
        "lora_A": (rng.standard_normal((E, R, D)) * 0.02).astype(np.float32),
        "lora_B": (rng.standard_normal((E, D, R)) * 0.02).astype(np.float32),
        "module_idx": 0,
        "k": 2,
    }
    y = kernel(**demo)
    print("out", y.shape, y.dtype, float(np.abs(y).max()))



# revision 2
# speedup vs baseline: 1.0674x; 1.0674x over previous
"""DynaLoRALinear Trainium2 kernel (v2).

Data-parallel over batch B across 8 NeuronCores (one sample per core).
Per core:
  - router:  logits = pooled @ (W_r @ gating_W).T  computed as a sharded
    partial (each core contracts over a 512-wide slice of D) + AllReduce.
  - gate weights from expert_scores ranks + module_prob>0.5 branch select.
  - base:    out = x_b @ W_base.T + b_base
  - lora:    tT = A_cat @ x_b.T, then out += tT.T @ (B_cat * gate)

All matmul operands are bf16 (same PE rate as tf32, half the DMA/SBUF):
  - x_b^T resident in SBUF (16 MB bf16), loaded once.
  - W_base^T streamed once as bf16 tiles through a ring pool deep enough
    that chunk c+1 prefetches while chunk c computes (no chunk barriers).
  - gate is ready before the main loop starts, so the rank-32 LoRA matmul
    folds into each PSUM accumulation group.
"""

import sys
import types

import numpy as np

B, L, D, E, R, NMOD = 8, 2048, 4096, 4, 8, 7
N_CORES = 8
DSH = D // N_CORES  # 512: per-core slice of D for the router shard
ER = E * R          # 32
KT = D // 128       # 32 k-tiles
MT = L // 128       # 16 m-tiles
NCH = D // 512      # 8 output-column chunks
MG = L // 512       # 4 m-groups for the lora-t pass


def _install_profile_hook():
    """Make bass_utils' trace path importable (no-op if already present)."""
    try:
        import antenv.axon_hooks  # noqa: F401
        return
    except ImportError:
        pass
    try:
        import antenv
    except ImportError:
        return
    mod = types.ModuleType("antenv.axon_hooks")
    mod._hook = None
    mod.set_axon_ntff_profile_hook = lambda h: setattr(mod, "_hook", h)
    mod.get_axon_ntff_profile_hook = lambda: mod._hook
    sys.modules["antenv.axon_hooks"] = mod
    antenv.axon_hooks = mod
    try:
        from trn_agent_boot.trn_boot import _ntff_profile_via_ctypes
        hook = _ntff_profile_via_ctypes("/opt/axon/libaxon_pjrt.so")
        if hook is not None:
            mod.set_axon_ntff_profile_hook(hook)
    except Exception:
        pass


_PROGRAM_CACHE = {}


def _build_program(k: int, module_idx: int, has_bias: bool):
    import concourse.mybir as mybir
    import concourse.tile as tile
    from concourse import bacc
    from concourse.masks import make_identity

    f32 = mybir.dt.float32
    bf16 = mybir.dt.bfloat16
    alu = mybir.AluOpType
    act_fn = mybir.ActivationFunctionType

    k_lo = max(1, k // 2)
    w_bufs = 28 if has_bias else 40

    nc = bacc.Bacc("TRN2", target_bir_lowering=False, debug=False,
                   num_devices=N_CORES)

    # --- DRAM I/O -------------------------------------------------------
    xT = nc.dram_tensor("xT", [D, L], bf16, kind="ExternalInput")
    WbT = nc.dram_tensor("WbT", [D, D], bf16, kind="ExternalInput")
    gw = nc.dram_tensor("gw", [D, DSH], bf16, kind="ExternalInput")
    WrT = nc.dram_tensor("WrT", [D, NMOD], bf16, kind="ExternalInput")
    pooledT = nc.dram_tensor("pooledT", [DSH, B], f32, kind="ExternalInput")
    scores_f = nc.dram_tensor("scores_f", [1, E * B], f32,
                              kind="ExternalInput")
    A_rhs = nc.dram_tensor("A_rhs", [D, ER], bf16, kind="ExternalInput")
    B_cat = nc.dram_tensor("B_cat", [ER, D], bf16, kind="ExternalInput")
    b_row = nc.dram_tensor("b_row", [1, D], f32, kind="ExternalInput")
    msel = nc.dram_tensor("msel", [ER, E * B], f32, kind="ExternalInput")
    out = nc.dram_tensor("out", [L, D], f32, kind="ExternalOutput")

    with tile.TileContext(nc) as tc:
        with (
            tc.tile_pool(name="const", bufs=1) as const_pool,
            tc.tile_pool(name="gatep", bufs=1) as gate_pool,
            tc.tile_pool(name="rsb", bufs=1) as rsb,
            tc.tile_pool(name="xsb", bufs=1) as xsb_pool,
            tc.tile_pool(name="wpool", bufs=w_bufs) as wpool,
            tc.tile_pool(name="apool", bufs=1) as apool,
            tc.tile_pool(name="tpool", bufs=1) as tpool,
            tc.tile_pool(name="ballp", bufs=1) as ball_pool,
            tc.tile_pool(name="bgp", bufs=NCH) as bg_pool,
            tc.tile_pool(name="epool", bufs=6) as epool,
            tc.tile_pool(name="biasp",
                         bufs=(NCH if has_bias else 1)) as biasp,
            tc.tile_pool(name="mps", bufs=8, space="PSUM") as mps,
            tc.tile_pool(name="rdram", bufs=1, space="DRAM") as rdram,
        ):
            ident = const_pool.tile([128, 128], f32)
            make_identity(nc, ident)
            gate32 = gate_pool.tile([ER, 1], f32)

            bias_all = []
            if has_bias:
                for hh in range(NCH):
                    bias_bc = biasp.tile([128, 512], f32, tag="biasbc",
                                         name=f"biasbc_{hh}")
                    nc.sync.dma_start(
                        bias_bc[0:1, :],
                        b_row[:, hh * 512:(hh + 1) * 512])
                    nc.gpsimd.partition_broadcast(bias_bc[:],
                                                  bias_bc[0:1, :])
                    bias_all.append(bias_bc)

            # ====== router part A: sharded W_r @ gating_W + AllReduce ===
            wr_sb = rsb.tile([128, KT, NMOD], bf16)
            nc.sync.dma_start(
                wr_sb[:], WrT[:].rearrange("(a p) m -> p a m", p=128))
            pt_sb = rsb.tile([128, 4, B], f32)
            nc.sync.dma_start(
                pt_sb[:], pooledT[:].rearrange("(a p) m -> p a m", p=128))
            msel_sb = rsb.tile([ER, E * B], f32)
            nc.sync.dma_start(msel_sb[:], msel[:])
            sc = rsb.tile([1, E * B], f32)
            nc.sync.dma_start(sc[:], scores_f[:])

            wc_ps = mps.tile([NMOD, DSH], f32, tag="ps", name="wc_ps")
            for kt in range(KT):
                gwt = wpool.tile([128, 512], bf16, tag="w",
                                 name=f"gwt_{kt}")
                nc.sync.dma_start(gwt[:], gw[kt * 128:(kt + 1) * 128, :])
                nc.tensor.matmul(wc_ps[:], wr_sb[:, kt, :], gwt[:],
                                 start=(kt == 0), stop=(kt == KT - 1))

            wc_sb = rsb.tile([NMOD, DSH], f32)
            nc.vector.tensor_copy(wc_sb[:], wc_ps[:])
            wct = rsb.tile([128, 4 * NMOD], f32)
            for j in range(4):
                tp = mps.tile([128, NMOD], f32, tag="ps", name=f"tp_{j}")
                nc.tensor.transpose(
                    tp[:], wc_sb[:, j * 128:(j + 1) * 128],
                    ident[0:NMOD, 0:NMOD])
                nc.vector.tensor_copy(
                    wct[:, j * NMOD:(j + 1) * NMOD], tp[:])

            lg_ps = mps.tile([NMOD, B], f32, tag="ps", name="lg_ps")
            for j in range(4):
                nc.tensor.matmul(lg_ps[:],
                                 wct[:, j * NMOD:(j + 1) * NMOD],
                                 pt_sb[:, j, :],
                                 start=(j == 0), stop=(j == 3))
            lp_sb = rsb.tile([NMOD, B], f32)
            nc.vector.tensor_copy(lp_sb[:], lg_ps[:])

            cc_in = rdram.tile([NMOD, B], f32)
            cc_out = rdram.tile([NMOD, B], f32)
            nc.gpsimd.dma_start(cc_in[:], lp_sb[:])
            nc.gpsimd.collective_compute(
                "AllReduce", alu.add,
                replica_groups=[list(range(N_CORES))],
                ins=[cc_in.opt()], outs=[cc_out.opt()])
            lg_sb = rsb.tile([NMOD, B], f32)
            nc.gpsimd.dma_start(lg_sb[:], cc_out[:])

            # ====== bulk input DMAs (x resident, A, B_cat) ==============
            a_sb = apool.tile([128, KT, ER], bf16)
            nc.sync.dma_start(
                a_sb[:], A_rhs[:].rearrange("(a p) m -> p a m", p=128))
            x_sb = xsb_pool.tile([128, KT, L], bf16)
            for kt in range(KT):
                nc.sync.dma_start(x_sb[:, kt, :],
                                  xT[kt * 128:(kt + 1) * 128, :])
            b_all = ball_pool.tile([ER, NCH, 512], bf16)
            nc.sync.dma_start(
                b_all[:], B_cat[:].rearrange("p (c n) -> p c n", c=NCH))

            # collective-independent: expert ranks from scores
            rank = rsb.tile([1, E * B], f32)
            nc.vector.memset(rank[:], 0.0)
            tmp = rsb.tile([1, B], f32)
            for e in range(E):
                re = rank[:, e * B:(e + 1) * B]
                se = sc[:, e * B:(e + 1) * B]
                for e2 in range(E):
                    if e2 == e:
                        continue
                    s2 = sc[:, e2 * B:(e2 + 1) * B]
                    nc.vector.tensor_tensor(tmp[:], s2, se, op=alu.is_gt)
                    nc.vector.tensor_add(re, re, tmp[:])
                    if e2 < e:
                        nc.vector.tensor_tensor(tmp[:], s2, se,
                                                op=alu.is_equal)
                        nc.vector.tensor_add(re, re, tmp[:])
            w_hi = rsb.tile([1, E * B], f32)
            nc.vector.tensor_scalar(w_hi[:], rank[:], float(k),
                                    1.0 / float(k),
                                    op0=alu.is_lt, op1=alu.mult)
            w_lo = rsb.tile([1, E * B], f32)
            nc.vector.tensor_scalar(w_lo[:], rank[:], float(k_lo),
                                    1.0 / float(k_lo),
                                    op0=alu.is_lt, op1=alu.mult)
            diff = rsb.tile([1, E * B], f32)
            nc.vector.tensor_sub(diff[:], w_hi[:], w_lo[:])

            # ====== lora-t: tT[er, m] = sum_k A[er, k] x[m, k] ==========
            tT = tpool.tile([ER, L], bf16)
            for mg in range(MG):
                ps_t = mps.tile([ER, 512], f32, tag="ps", name=f"pst_{mg}")
                for kt in range(KT):
                    nc.tensor.matmul(ps_t[:], a_sb[:, kt, :],
                                     x_sb[:, kt, mg * 512:(mg + 1) * 512],
                                     start=(kt == 0), stop=(kt == KT - 1))
                nc.vector.tensor_copy(tT[:, mg * 512:(mg + 1) * 512],
                                      ps_t[:])

            # ====== router part B: softmax branch -> per-core gate ======
            ltp = mps.tile([B, NMOD], f32, tag="ps", name="ltp")
            nc.tensor.transpose(ltp[:], lg_sb[:], ident[0:NMOD, 0:NMOD])
            lt = rsb.tile([B, NMOD], f32)
            nc.vector.tensor_copy(lt[:], ltp[:])
            mx = rsb.tile([B, 1], f32)
            nc.vector.tensor_reduce(out=mx[:], in_=lt[:], op=alu.max,
                                    axis=mybir.AxisListType.X)
            mxn = rsb.tile([B, 1], f32)
            nc.vector.tensor_scalar_mul(mxn[:], mx[:], -1.0)
            ex = rsb.tile([B, NMOD], f32)
            nc.scalar.activation(ex[:], lt[:], act_fn.Exp, bias=mxn[:])
            sm = rsb.tile([B, 1], f32)
            nc.vector.tensor_reduce(out=sm[:], in_=ex[:], op=alu.add,
                                    axis=mybir.AxisListType.X)
            rs = rsb.tile([B, 1], f32)
            nc.vector.reciprocal(rs[:], sm[:])
            p0 = rsb.tile([B, 1], f32)
            nc.vector.tensor_mul(
                p0[:], ex[:, module_idx:module_idx + 1], rs[:])
            hi = rsb.tile([B, 1], f32)
            nc.vector.tensor_single_scalar(hi[:], p0[:], 0.5, alu.is_gt)
            hp = mps.tile([1, B], f32, tag="ps", name="hp")
            nc.tensor.transpose(hp[:], hi[:], ident[0:B, 0:B])
            hi_row = rsb.tile([1, B], f32)
            nc.vector.tensor_copy(hi_row[:], hp[:])
            gate = rsb.tile([1, E * B], f32)
            for e in range(E):
                nc.vector.tensor_mul(gate[:, e * B:(e + 1) * B],
                                     diff[:, e * B:(e + 1) * B],
                                     hi_row[:])
            nc.vector.tensor_add(gate[:], gate[:], w_lo[:])
            gateb = rsb.tile([ER, E * B], f32)
            nc.gpsimd.partition_broadcast(gateb[:], gate[:])
            g32m = rsb.tile([ER, E * B], f32)
            nc.vector.tensor_tensor(g32m[:], gateb[:], msel_sb[:],
                                    op=alu.mult)
            nc.vector.tensor_reduce(out=gate32[:], in_=g32m[:],
                                    op=alu.add,
                                    axis=mybir.AxisListType.X)

            # gate-scaled B tiles, one per output chunk (gate known now)
            bg_tiles = []
            for c in range(NCH):
                bg = bg_pool.tile([ER, 512], bf16, tag="bg",
                                  name=f"bg_{c}")
                nc.vector.tensor_scalar_mul(bg[:], b_all[:, c, :],
                                            gate32[:, 0:1])
                bg_tiles.append(bg)

            # ============== main: base + lora ==========================
            for c in range(NCH):
                wt_c = []
                for kt in range(KT):
                    wt = wpool.tile([128, 512], bf16, tag="w",
                                    name=f"w_{c}_{kt}")
                    nc.sync.dma_start(
                        wt[:],
                        WbT[kt * 128:(kt + 1) * 128,
                            c * 512:(c + 1) * 512])
                    wt_c.append(wt)
                for mt in range(MT):
                    ps = mps.tile([128, 512], f32, tag="ps",
                                  name=f"ps_{c}_{mt}")
                    for kt in range(KT):
                        nc.tensor.matmul(
                            ps[:], x_sb[:, kt, mt * 128:(mt + 1) * 128],
                            wt_c[kt][:], start=(kt == 0), stop=False)
                    nc.tensor.matmul(ps[:], tT[:, mt * 128:(mt + 1) * 128],
                                     bg_tiles[c][:],
                                     start=False, stop=True)
                    ev = epool.tile([128, 512], f32, tag="ev",
                                    name=f"ev_{c}_{mt}")
                    if has_bias:
                        nc.vector.tensor_add(ev[:], ps[:], bias_all[c][:])
                    elif mt % 2 == 0:
                        nc.vector.tensor_copy(ev[:], ps[:])
                    else:
                        nc.scalar.activation(ev[:], ps[:], act_fn.Copy)
                    nc.sync.dma_start(
                        out[mt * 128:(mt + 1) * 128,
                            c * 512:(c + 1) * 512],
                        ev[:])

    nc.compile()
    return nc


def kernel(**inputs) -> np.ndarray:
    _install_profile_hook()
    import ml_dtypes
    bf = ml_dtypes.bfloat16

    x = np.asarray(inputs["x"], dtype=np.float32)
    expert_scores = np.asarray(inputs["expert_scores"], dtype=np.float32)
    W_base = np.asarray(inputs["W_base"], dtype=np.float32)
    b_base = np.asarray(inputs["b_base"], dtype=np.float32)
    gating_W = np.asarray(inputs["gating_W"], dtype=np.float32)
    W_r = np.asarray(inputs["W_r"], dtype=np.float32)
    lora_A = np.asarray(inputs["lora_A"], dtype=np.float32)
    lora_B = np.asarray(inputs["lora_B"], dtype=np.float32)
    module_idx = int(np.asarray(inputs["module_idx"]))
    k = int(np.asarray(inputs["k"]))

    has_bias = bool(np.any(b_base != 0.0))
    key = (k, module_idx, has_bias)
    if key not in _PROGRAM_CACHE:
        _PROGRAM_CACHE[key] = _build_program(k, module_idx, has_bias)
    nc = _PROGRAM_CACHE[key]

    # --- host-side layout prep (transposes/slices/bf16 rounding) --------
    x_bf = x.astype(bf)                                  # [B, L, D]
    WbT_np = np.ascontiguousarray(W_base.T).astype(bf)   # [D, D]
    WrT_np = np.ascontiguousarray(W_r.T).astype(bf)      # [D, NMOD]
    A_np = np.ascontiguousarray(
        lora_A.reshape(ER, D).T).astype(bf)              # [D, ER]
    B_np = np.ascontiguousarray(
        lora_B.transpose(0, 2, 1).reshape(ER, D)).astype(bf)  # [ER, D]
    scores_f_np = np.ascontiguousarray(
        expert_scores.T.reshape(1, E * B))               # [1, E*B]
    b_row_np = b_base.reshape(1, D)
    pooled = x[:, -1, :]                                 # [B, D] fp32

    in_maps = []
    for c in range(N_CORES):
        msel_np = np.zeros((ER, E, B), dtype=np.float32)
        for p in range(ER):
            msel_np[p, p // R, c] = 1.0
        msel_np = msel_np.reshape(ER, E * B)
        in_maps.append({
            "xT": np.ascontiguousarray(x_bf[c].T),
            "WbT": WbT_np,
            "gw": np.ascontiguousarray(
                gating_W[:, c * DSH:(c + 1) * DSH]).astype(bf),
            "WrT": WrT_np,
            "pooledT": np.ascontiguousarray(
                pooled[:, c * DSH:(c + 1) * DSH].T),
            "scores_f": scores_f_np,
            "A_rhs": A_np,
            "B_cat": B_np,
            "b_row": b_row_np,
            "msel": msel_np,
        })

    from concourse.bass_utils import run_bass_kernel_spmd
    res = run_bass_kernel_spmd(nc, in_maps, core_ids=list(range(N_CORES)))
    return np.stack([res.results[c]["out"] for c in range(N_CORES)], axis=0)


if __name__ == "__main__":
    rng = np.random.default_rng(0)
    demo = {
        "x": (rng.standard_normal((B, L, D)) * 0.02).astype(np.float32),
        "expert_scores": rng.random((B, E), dtype=np.float32),
        "W_base": (rng.standard_normal((D, D)) * 0.02).astype(np.float32),
        "b_base": np.zeros(D, np.float32),
        "gating_W": (rng.standard_normal((D, D)) * 0.02).astype(np.float32),
        "W_r": (rng.standard_normal((NMOD, D)) * 0.02).astype(np.float32),
        "lora_A": (rng.standard_normal((E, R, D)) * 0.02).astype(np.float32),
        "lora_B": (rng.standard_normal((E, D, R)) * 0.02).astype(np.float32),
        "module_idx": 0,
        "k": 2,
    }
    y = kernel(**demo)
    print("out", y.shape, y.dtype, float(np.abs(y).max()))


# revision 3
# speedup vs baseline: 1.3015x; 1.2193x over previous
"""DynaLoRALinear Trainium2 kernel (v3).

Data-parallel over batch B across 8 NeuronCores (one sample per core).
Per core:
  - router: logits = pooled @ C.T with C = W_r @ gating_W folded on the
    host (weight-only reassociation), so every core computes the full
    [NMOD, B] logits locally -- no collective at all.
  - gate weights from expert_scores ranks + module_prob>0.5 branch select.
  - base:   out = x_b @ W_base.T + b_base
  - lora:   tT = A_cat @ x_b.T, then out += tT.T @ (B_cat * gate)

All matmul operands are bf16. x_b^T is SBUF-resident (16 MB, 32 separate
k-tiles so compute can pace the incoming DMA stream), W_base^T streams
once through a ring pool (pre-tiled in DRAM, contiguous 128 KB tiles).
Phase A runs chunk 0 k-outer across 8 PSUM banks so the PE stays busy
while x streams in; chunk 0's LoRA term is applied later via an SWDGE
read-modify-write pass once the gate is known.
"""

import sys
import types

import numpy as np

B, L, D, E, R, NMOD = 8, 2048, 4096, 4, 8, 7
N_CORES = 8
ER = E * R          # 32
KT = D // 128       # 32 k-tiles
MT = L // 128       # 16 m-tiles
NCH = D // 512      # 8 output-column chunks
MG = L // 512       # 4 m-groups for the lora-t pass
PH_A = 7            # chunk-0 m-tiles processed k-outer during x stream


def _install_profile_hook():
    """Make bass_utils' trace path importable (no-op if already present)."""
    try:
        import antenv.axon_hooks  # noqa: F401
        return
    except ImportError:
        pass
    try:
        import antenv
    except ImportError:
        return
    mod = types.ModuleType("antenv.axon_hooks")
    mod._hook = None
    mod.set_axon_ntff_profile_hook = lambda h: setattr(mod, "_hook", h)
    mod.get_axon_ntff_profile_hook = lambda: mod._hook
    sys.modules["antenv.axon_hooks"] = mod
    antenv.axon_hooks = mod
    try:
        from trn_agent_boot.trn_boot import _ntff_profile_via_ctypes
        hook = _ntff_profile_via_ctypes("/opt/axon/libaxon_pjrt.so")
        if hook is not None:
            mod.set_axon_ntff_profile_hook(hook)
    except Exception:
        pass


_PROGRAM_CACHE = {}


def _build_program(k: int, module_idx: int, has_bias: bool):
    import concourse.mybir as mybir
    import concourse.tile as tile
    from concourse import bacc
    from concourse.masks import make_identity

    f32 = mybir.dt.float32
    bf16 = mybir.dt.bfloat16
    alu = mybir.AluOpType
    act_fn = mybir.ActivationFunctionType

    k_lo = max(1, k // 2)
    w_bufs = 28 if has_bias else 40

    nc = bacc.Bacc("TRN2", target_bir_lowering=False, debug=False,
                   num_devices=N_CORES)

    # --- DRAM I/O -------------------------------------------------------
    xT = nc.dram_tensor("xT", [D, L], bf16, kind="ExternalInput")
    Wt = nc.dram_tensor("Wt", [NCH, KT, 128, 512], bf16,
                        kind="ExternalInput")
    ctT = nc.dram_tensor("ctT", [D, NMOD], bf16, kind="ExternalInput")
    pooledT = nc.dram_tensor("pooledT", [D, B], bf16, kind="ExternalInput")
    scores_f = nc.dram_tensor("scores_f", [1, E * B], f32,
                              kind="ExternalInput")
    A_rhs = nc.dram_tensor("A_rhs", [D, ER], bf16, kind="ExternalInput")
    B_cat = nc.dram_tensor("B_cat", [ER, D], bf16, kind="ExternalInput")
    b_row = nc.dram_tensor("b_row", [1, D], f32, kind="ExternalInput")
    msel = nc.dram_tensor("msel", [ER, E * B], f32, kind="ExternalInput")
    out = nc.dram_tensor("out", [L, D], f32, kind="ExternalOutput")

    with tile.TileContext(nc) as tc:
        with (
            tc.tile_pool(name="const", bufs=1) as const_pool,
            tc.tile_pool(name="gatep", bufs=1) as gate_pool,
            tc.tile_pool(name="rsb", bufs=1) as rsb,
            tc.tile_pool(name="xsb", bufs=KT) as xsb_pool,
            tc.tile_pool(name="wpool", bufs=w_bufs) as wpool,
            tc.tile_pool(name="apool", bufs=1) as apool,
            tc.tile_pool(name="tpool", bufs=1) as tpool,
            tc.tile_pool(name="ballp", bufs=1) as ball_pool,
            tc.tile_pool(name="bgp", bufs=NCH) as bg_pool,
            tc.tile_pool(name="epool", bufs=6) as epool,
            tc.tile_pool(name="biasp",
                         bufs=(NCH if has_bias else 1)) as biasp,
            tc.tile_pool(name="mps", bufs=8, space="PSUM") as mps,
        ):
            ident = const_pool.tile([128, 128], f32)
            make_identity(nc, ident)
            gate32 = gate_pool.tile([ER, 1], f32)

            bias_all = []
            if has_bias:
                for hh in range(NCH):
                    bias_bc = biasp.tile([128, 512], f32, tag="biasbc",
                                         name=f"biasbc_{hh}")
                    nc.sync.dma_start(
                        bias_bc[0:1, :],
                        b_row[:, hh * 512:(hh + 1) * 512])
                    nc.gpsimd.partition_broadcast(bias_bc[:],
                                                  bias_bc[0:1, :])
                    bias_all.append(bias_bc)

            # ====== small input DMAs ===================================
            ct_sb = rsb.tile([128, KT, NMOD], bf16)
            nc.sync.dma_start(
                ct_sb[:], ctT[:].rearrange("(a p) m -> p a m", p=128))
            pt_sb = rsb.tile([128, KT, B], bf16)
            nc.sync.dma_start(
                pt_sb[:], pooledT[:].rearrange("(a p) m -> p a m", p=128))
            msel_sb = rsb.tile([ER, E * B], f32)
            nc.sync.dma_start(msel_sb[:], msel[:])
            sc = rsb.tile([1, E * B], f32)
            nc.sync.dma_start(sc[:], scores_f[:])
            a_sb = apool.tile([128, KT, ER], bf16)
            nc.sync.dma_start(
                a_sb[:], A_rhs[:].rearrange("(a p) m -> p a m", p=128))
            b_all = ball_pool.tile([ER, NCH, 512], bf16)
            nc.sync.dma_start(
                b_all[:], B_cat[:].rearrange("p (c n) -> p c n", c=NCH))

            # ====== router: logits = pooled @ C.T (local, no collective)
            ps_r = mps.tile([NMOD, B], f32, tag="ps", name="ps_r")
            for kt in range(KT):
                nc.tensor.matmul(ps_r[:], ct_sb[:, kt, :], pt_sb[:, kt, :],
                                 start=(kt == 0), stop=(kt == KT - 1))
            lr_sb = rsb.tile([NMOD, B], f32)
            nc.vector.tensor_copy(lr_sb[:], ps_r[:])

            # collective-independent: expert ranks from scores
            rank = rsb.tile([1, E * B], f32)
            nc.vector.memset(rank[:], 0.0)
            tmp = rsb.tile([1, B], f32)
            for e in range(E):
                re = rank[:, e * B:(e + 1) * B]
                se = sc[:, e * B:(e + 1) * B]
                for e2 in range(E):
                    if e2 == e:
                        continue
                    s2 = sc[:, e2 * B:(e2 + 1) * B]
                    nc.vector.tensor_tensor(tmp[:], s2, se, op=alu.is_gt)
                    nc.vector.tensor_add(re, re, tmp[:])
                    if e2 < e:
                        nc.vector.tensor_tensor(tmp[:], s2, se,
                                                op=alu.is_equal)
                        nc.vector.tensor_add(re, re, tmp[:])
            w_hi = rsb.tile([1, E * B], f32)
            nc.vector.tensor_scalar(w_hi[:], rank[:], float(k),
                                    1.0 / float(k),
                                    op0=alu.is_lt, op1=alu.mult)
            w_lo = rsb.tile([1, E * B], f32)
            nc.vector.tensor_scalar(w_lo[:], rank[:], float(k_lo),
                                    1.0 / float(k_lo),
                                    op0=alu.is_lt, op1=alu.mult)
            diff = rsb.tile([1, E * B], f32)
            nc.vector.tensor_sub(diff[:], w_hi[:], w_lo[:])

            # ====== router part B: softmax branch -> per-core gate ======
            ltp = mps.tile([B, NMOD], f32, tag="ps", name="ltp")
            nc.tensor.transpose(ltp[:], lr_sb[:], ident[0:NMOD, 0:NMOD])
            lt = rsb.tile([B, NMOD], f32)
            nc.vector.tensor_copy(lt[:], ltp[:])
            mx = rsb.tile([B, 1], f32)
            nc.vector.tensor_reduce(out=mx[:], in_=lt[:], op=alu.max,
                                    axis=mybir.AxisListType.X)
            mxn = rsb.tile([B, 1], f32)
            nc.vector.tensor_scalar_mul(mxn[:], mx[:], -1.0)
            ex = rsb.tile([B, NMOD], f32)
            nc.scalar.activation(ex[:], lt[:], act_fn.Exp, bias=mxn[:])
            sm = rsb.tile([B, 1], f32)
            nc.vector.tensor_reduce(out=sm[:], in_=ex[:], op=alu.add,
                                    axis=mybir.AxisListType.X)
            rs = rsb.tile([B, 1], f32)
            nc.vector.reciprocal(rs[:], sm[:])
            p0 = rsb.tile([B, 1], f32)
            nc.vector.tensor_mul(
                p0[:], ex[:, module_idx:module_idx + 1], rs[:])
            hi = rsb.tile([B, 1], f32)
            nc.vector.tensor_single_scalar(hi[:], p0[:], 0.5, alu.is_gt)
            hp = mps.tile([1, B], f32, tag="ps", name="hp")
            nc.tensor.transpose(hp[:], hi[:], ident[0:B, 0:B])
            hi_row = rsb.tile([1, B], f32)
            nc.vector.tensor_copy(hi_row[:], hp[:])
            gate = rsb.tile([1, E * B], f32)
            for e in range(E):
                nc.vector.tensor_mul(gate[:, e * B:(e + 1) * B],
                                     diff[:, e * B:(e + 1) * B],
                                     hi_row[:])
            nc.vector.tensor_add(gate[:], gate[:], w_lo[:])
            gateb = rsb.tile([ER, E * B], f32)
            nc.gpsimd.partition_broadcast(gateb[:], gate[:])
            g32m = rsb.tile([ER, E * B], f32)
            nc.vector.tensor_tensor(g32m[:], gateb[:], msel_sb[:],
                                    op=alu.mult)
            nc.vector.tensor_reduce(out=gate32[:], in_=g32m[:],
                                    op=alu.add,
                                    axis=mybir.AxisListType.X)

            # gate-scaled B tiles, one per output chunk
            bg_tiles = []
            for c in range(NCH):
                bg = bg_pool.tile([ER, 512], bf16, tag="bg",
                                  name=f"bg_{c}")
                nc.vector.tensor_scalar_mul(bg[:], b_all[:, c, :],
                                            gate32[:, 0:1])
                bg_tiles.append(bg)

            # ====== phase A: chunk 0 k-outer while x streams in =========
            x_tiles = []
            wt0 = []
            psA = [mps.tile([128, 512], f32, tag="ps", name=f"psA_{mt}")
                   for mt in range(PH_A)]
            ps_t0 = mps.tile([ER, 512], f32, tag="ps", name="pst_0")
            tT = tpool.tile([ER, L], bf16)
            for kt in range(KT):
                xs = xsb_pool.tile([128, L], bf16, tag="x",
                                   name=f"x_{kt}")
                nc.sync.dma_start(xs[:], xT[kt * 128:(kt + 1) * 128, :])
                x_tiles.append(xs)
                wt = wpool.tile([128, 512], bf16, tag="w",
                                name=f"w_0_{kt}")
                nc.sync.dma_start(wt[:], Wt[0, kt])
                wt0.append(wt)
                st, sp = (kt == 0), (kt == KT - 1)
                nc.tensor.matmul(ps_t0[:], a_sb[:, kt, :], xs[:, 0:512],
                                 start=st, stop=sp)
                for mt in range(PH_A):
                    nc.tensor.matmul(psA[mt][:],
                                     xs[:, mt * 128:(mt + 1) * 128],
                                     wt[:], start=st, stop=sp)
            nc.vector.tensor_copy(tT[:, 0:512], ps_t0[:])
            for mt in range(PH_A):
                ev = epool.tile([128, 512], f32, tag="ev",
                                name=f"evA_{mt}")
                if has_bias:
                    nc.vector.tensor_add(ev[:], psA[mt][:],
                                         bias_all[0][:])
                elif mt % 2 == 0:
                    nc.vector.tensor_copy(ev[:], psA[mt][:])
                else:
                    nc.scalar.activation(ev[:], psA[mt][:], act_fn.Copy)
                nc.sync.dma_start(
                    out[mt * 128:(mt + 1) * 128, 0:512], ev[:])

            # ====== phase B: lora-t mg1..3 + rest of chunk 0 ===========
            for mg in range(1, MG):
                ps_t = mps.tile([ER, 512], f32, tag="ps", name=f"pst_{mg}")
                for kt in range(KT):
                    nc.tensor.matmul(
                        ps_t[:], a_sb[:, kt, :],
                        x_tiles[kt][:, mg * 512:(mg + 1) * 512],
                        start=(kt == 0), stop=(kt == KT - 1))
                nc.vector.tensor_copy(tT[:, mg * 512:(mg + 1) * 512],
                                      ps_t[:])
            for mt in range(PH_A, MT):
                ps = mps.tile([128, 512], f32, tag="ps", name=f"ps_0_{mt}")
                for kt in range(KT):
                    nc.tensor.matmul(
                        ps[:], x_tiles[kt][:, mt * 128:(mt + 1) * 128],
                        wt0[kt][:], start=(kt == 0), stop=(kt == KT - 1))
                ev = epool.tile([128, 512], f32, tag="ev",
                                name=f"ev_0_{mt}")
                if has_bias:
                    nc.vector.tensor_add(ev[:], ps[:], bias_all[0][:])
                elif mt % 2 == 0:
                    nc.vector.tensor_copy(ev[:], ps[:])
                else:
                    nc.scalar.activation(ev[:], ps[:], act_fn.Copy)
                nc.sync.dma_start(
                    out[mt * 128:(mt + 1) * 128, 0:512], ev[:])

            # ====== phase C: chunks 1..7 (lora MM first, then base) =====
            def emit_chunk(c):
                wt_c = []
                for kt in range(KT):
                    wt = wpool.tile([128, 512], bf16, tag="w",
                                    name=f"w_{c}_{kt}")
                    nc.sync.dma_start(wt[:], Wt[c, kt])
                    wt_c.append(wt)
                for mt in range(MT):
                    ps = mps.tile([128, 512], f32, tag="ps",
                                  name=f"ps_{c}_{mt}")
                    nc.tensor.matmul(ps[:], tT[:, mt * 128:(mt + 1) * 128],
                                     bg_tiles[c][:],
                                     start=True, stop=False)
                    for kt in range(KT):
                        nc.tensor.matmul(
                            ps[:], x_tiles[kt][:, mt * 128:(mt + 1) * 128],
                            wt_c[kt][:], start=False, stop=(kt == KT - 1))
                    ev = epool.tile([128, 512], f32, tag="ev",
                                    name=f"ev_{c}_{mt}")
                    if has_bias:
                        nc.vector.tensor_add(ev[:], ps[:], bias_all[c][:])
                    elif mt % 2 == 0:
                        nc.vector.tensor_copy(ev[:], ps[:])
                    else:
                        nc.scalar.activation(ev[:], ps[:], act_fn.Copy)
                    nc.sync.dma_start(
                        out[mt * 128:(mt + 1) * 128,
                            c * 512:(c + 1) * 512],
                        ev[:])

            emit_chunk(1)

            # ====== phase D: deferred chunk-0 lora via SWDGE accum ======
            for mt in range(MT):
                lp = mps.tile([128, 512], f32, tag="ps", name=f"lp_{mt}")
                nc.tensor.matmul(lp[:], tT[:, mt * 128:(mt + 1) * 128],
                                 bg_tiles[0][:], start=True, stop=True)
                lev = epool.tile([128, 512], f32, tag="ev",
                                 name=f"lev_{mt}")
                nc.vector.tensor_copy(lev[:], lp[:])
                nc.gpsimd.dma_start(
                    out[mt * 128:(mt + 1) * 128, 0:512], lev[:],
                    accum_op=alu.add)

            for c in range(2, NCH):
                emit_chunk(c)

    nc.compile()
    return nc


def kernel(**inputs) -> np.ndarray:
    _install_profile_hook()
    import ml_dtypes
    bf = ml_dtypes.bfloat16

    x = np.asarray(inputs["x"], dtype=np.float32)
    expert_scores = np.asarray(inputs["expert_scores"], dtype=np.float32)
    W_base = np.asarray(inputs["W_base"], dtype=np.float32)
    b_base = np.asarray(inputs["b_base"], dtype=np.float32)
    gating_W = np.asarray(inputs["gating_W"], dtype=np.float32)
    W_r = np.asarray(inputs["W_r"], dtype=np.float32)
    lora_A = np.asarray(inputs["lora_A"], dtype=np.float32)
    lora_B = np.asarray(inputs["lora_B"], dtype=np.float32)
    module_idx = int(np.asarray(inputs["module_idx"]))
    k = int(np.asarray(inputs["k"]))

    has_bias = bool(np.any(b_base != 0.0))
    key = (k, module_idx, has_bias)
    if key not in _PROGRAM_CACHE:
        _PROGRAM_CACHE[key] = _build_program(k, module_idx, has_bias)
    nc = _PROGRAM_CACHE[key]

    # --- host-side layout prep (transposes/fold/bf16 rounding) ----------
    x_bf = x.astype(bf)                                  # [B, L, D]
    Wt_np = np.ascontiguousarray(
        W_base.T.reshape(KT, 128, NCH, 512).transpose(2, 0, 1, 3)
    ).astype(bf)                                         # [NCH,KT,128,512]
    C = W_r @ gating_W                                   # [NMOD, D] fp32
    ctT_np = np.ascontiguousarray(C.T).astype(bf)        # [D, NMOD]
    A_np = np.ascontiguousarray(
        lora_A.reshape(ER, D).T).astype(bf)              # [D, ER]
    B_np = np.ascontiguousarray(
        lora_B.transpose(0, 2, 1).reshape(ER, D)).astype(bf)  # [ER, D]
    scores_f_np = np.ascontiguousarray(
        expert_scores.T.reshape(1, E * B))               # [1, E*B]
    b_row_np = b_base.reshape(1, D)
    pooledT_np = np.ascontiguousarray(x[:, -1, :].T).astype(bf)  # [D, B]

    in_maps = []
    for c in range(N_CORES):
        msel_np = np.zeros((ER, E, B), dtype=np.float32)
        for p in range(ER):
            msel_np[p, p // R, c] = 1.0
        msel_np = msel_np.reshape(ER, E * B)
        in_maps.append({
            "xT": np.ascontiguousarray(x_bf[c].T),
            "Wt": Wt_np,
            "ctT": ctT_np,
            "pooledT": pooledT_np,
            "scores_f": scores_f_np,
            "A_rhs": A_np,
            "B_cat": B_np,
            "b_row": b_row_np,
            "msel": msel_np,
        })

    from concourse.bass_utils import run_bass_kernel_spmd
    res = run_bass_kernel_spmd(nc, in_maps, core_ids=list(range(N_CORES)))
    return np.stack([res.results[c]["out"] for c in range(N_CORES)], axis=0)


if __name__ == "__main__":
    rng = np.random.default_rng(0)
    demo = {
        "x": (rng.standard_normal((B, L, D)) * 0.02).astype(np.float32),
        "expert_scores": rng.random((B, E), dtype=np.float32),
        "W_base": (rng.standard_normal((D, D)) * 0.02).astype(np.float32),
        "b_base": np.zeros(D, np.float32),
        "gating_W": (rng.standard_normal((D, D)) * 0.02).astype(np.float32),
        "W_r": (rng.standard_normal((NMOD, D)) * 0.02).astype(np.float32),
        "lora_A": (rng.standard_normal((E, R, D)) * 0.02).astype(np.float32),
        "lora_B": (rng.standard_normal((E, D, R)) * 0.02).astype(np.float32),
        "module_idx": 0,
        "k": 2,
    }
    y = kernel(**demo)
    print("out", y.shape, y.dtype, float(np.abs(y).max()))


# revision 8
# speedup vs baseline: 1.3714x; 1.0537x over previous
"""DynaLoRALinear Trainium2 kernel (v3).

Data-parallel over batch B across 8 NeuronCores (one sample per core).
Per core:
  - router: logits = pooled @ C.T with C = W_r @ gating_W folded on the
    host (weight-only reassociation), so every core computes the full
    [NMOD, B] logits locally -- no collective at all.
  - gate weights from expert_scores ranks + module_prob>0.5 branch select.
  - base:   out = x_b @ W_base.T + b_base
  - lora:   tT = A_cat @ x_b.T, then out += tT.T @ (B_cat * gate)

All matmul operands are bf16. x_b^T is SBUF-resident (16 MB, 32 separate
k-tiles so compute can pace the incoming DMA stream), W_base^T streams
once through a ring pool (pre-tiled in DRAM, contiguous 128 KB tiles).
Phase A runs chunk 0 k-outer across 8 PSUM banks so the PE stays busy
while x streams in; chunk 0's LoRA term is applied later via an SWDGE
read-modify-write pass once the gate is known.
"""

import sys
import types

import numpy as np

B, L, D, E, R, NMOD = 8, 2048, 4096, 4, 8, 7
N_CORES = 8
ER = E * R          # 32
KT = D // 128       # 32 k-tiles
MT = L // 128       # 16 m-tiles
NCH = D // 512      # 8 output-column chunks
MG = L // 512       # 4 m-groups for the lora-t pass
PH_A = 7            # chunk-0 m-tiles processed k-outer during x stream


def _install_profile_hook():
    """Make bass_utils' trace path importable (no-op if already present)."""
    try:
        import antenv.axon_hooks  # noqa: F401
        return
    except ImportError:
        pass
    try:
        import antenv
    except ImportError:
        return
    mod = types.ModuleType("antenv.axon_hooks")
    mod._hook = None
    mod.set_axon_ntff_profile_hook = lambda h: setattr(mod, "_hook", h)
    mod.get_axon_ntff_profile_hook = lambda: mod._hook
    sys.modules["antenv.axon_hooks"] = mod
    antenv.axon_hooks = mod
    try:
        from trn_agent_boot.trn_boot import _ntff_profile_via_ctypes
        hook = _ntff_profile_via_ctypes("/opt/axon/libaxon_pjrt.so")
        if hook is not None:
            mod.set_axon_ntff_profile_hook(hook)
    except Exception:
        pass


_PROGRAM_CACHE = {}


def _build_program(k: int, module_idx: int, has_bias: bool):
    import concourse.mybir as mybir
    import concourse.tile as tile
    from concourse import bacc
    from concourse.masks import make_identity

    f32 = mybir.dt.float32
    bf16 = mybir.dt.bfloat16
    alu = mybir.AluOpType
    act_fn = mybir.ActivationFunctionType

    k_lo = max(1, k // 2)
    w_bufs = 36 if has_bias else 52

    nc = bacc.Bacc("TRN2", target_bir_lowering=False, debug=False,
                   num_devices=N_CORES)

    # --- DRAM I/O -------------------------------------------------------
    xT = nc.dram_tensor("xT", [D, L], bf16, kind="ExternalInput")
    Wt = nc.dram_tensor("Wt", [NCH, KT, 128, 512], bf16,
                        kind="ExternalInput")
    ctT = nc.dram_tensor("ctT", [D, NMOD], bf16, kind="ExternalInput")
    pooledT = nc.dram_tensor("pooledT", [D, B], bf16, kind="ExternalInput")
    scores_f = nc.dram_tensor("scores_f", [1, E * B], f32,
                              kind="ExternalInput")
    A_rhs = nc.dram_tensor("A_rhs", [D, ER], bf16, kind="ExternalInput")
    B_cat = nc.dram_tensor("B_cat", [ER, D], bf16, kind="ExternalInput")
    b_row = nc.dram_tensor("b_row", [1, D], f32, kind="ExternalInput")
    msel = nc.dram_tensor("msel", [ER, E * B], f32, kind="ExternalInput")
    out = nc.dram_tensor("out", [L, D], f32, kind="ExternalOutput")

    with tile.TileContext(nc) as tc:
        with (
            tc.tile_pool(name="const", bufs=1) as const_pool,
            tc.tile_pool(name="gatep", bufs=1) as gate_pool,
            tc.tile_pool(name="rsb", bufs=1) as rsb,
            tc.tile_pool(name="xsb", bufs=KT) as xsb_pool,
            tc.tile_pool(name="wpool", bufs=w_bufs) as wpool,
            tc.tile_pool(name="apool", bufs=1) as apool,
            tc.tile_pool(name="tpool", bufs=1) as tpool,
            tc.tile_pool(name="ballp", bufs=1) as ball_pool,
            tc.tile_pool(name="epool", bufs=4) as epool,
            tc.tile_pool(name="biasp",
                         bufs=(NCH if has_bias else 1)) as biasp,
            tc.tile_pool(name="mps", bufs=8, space="PSUM") as mps,
        ):
            ident = const_pool.tile([128, 128], f32)
            make_identity(nc, ident)
            gate32 = gate_pool.tile([ER, 1], f32)

            bias_all = []
            if has_bias:
                for hh in range(NCH):
                    bias_bc = biasp.tile([128, 512], f32, tag="biasbc",
                                         name=f"biasbc_{hh}")
                    nc.sync.dma_start(
                        bias_bc[0:1, :],
                        b_row[:, hh * 512:(hh + 1) * 512])
                    nc.gpsimd.partition_broadcast(bias_bc[:],
                                                  bias_bc[0:1, :])
                    bias_all.append(bias_bc)

            # ====== small input DMAs ===================================
            ct_sb = rsb.tile([128, KT, NMOD], bf16)
            nc.sync.dma_start(
                ct_sb[:], ctT[:].rearrange("(a p) m -> p a m", p=128))
            pt_sb = rsb.tile([128, KT, B], bf16)
            nc.sync.dma_start(
                pt_sb[:], pooledT[:].rearrange("(a p) m -> p a m", p=128))
            msel_sb = rsb.tile([ER, E * B], f32)
            nc.sync.dma_start(msel_sb[:], msel[:])
            sc = rsb.tile([1, E * B], f32)
            nc.sync.dma_start(sc[:], scores_f[:])
            a_sb = apool.tile([128, KT, ER], bf16)
            nc.sync.dma_start(
                a_sb[:], A_rhs[:].rearrange("(a p) m -> p a m", p=128))
            b_all = ball_pool.tile([ER, NCH, 512], bf16)
            nc.sync.dma_start(
                b_all[:], B_cat[:].rearrange("p (c n) -> p c n", c=NCH))

            # ====== router: logits = pooled @ C.T (local, no collective)
            ps_r = mps.tile([NMOD, B], f32, tag="ps", name="ps_r")
            for kt in range(KT):
                nc.tensor.matmul(ps_r[:], ct_sb[:, kt, :], pt_sb[:, kt, :],
                                 start=(kt == 0), stop=(kt == KT - 1))
            lr_sb = rsb.tile([NMOD, B], f32)
            nc.vector.tensor_copy(lr_sb[:], ps_r[:])

            # collective-independent: expert ranks from scores
            rank = rsb.tile([1, E * B], f32)
            nc.vector.memset(rank[:], 0.0)
            tmp = rsb.tile([1, B], f32)
            for e in range(E):
                re = rank[:, e * B:(e + 1) * B]
                se = sc[:, e * B:(e + 1) * B]
                for e2 in range(E):
                    if e2 == e:
                        continue
                    s2 = sc[:, e2 * B:(e2 + 1) * B]
                    nc.vector.tensor_tensor(tmp[:], s2, se, op=alu.is_gt)
                    nc.vector.tensor_add(re, re, tmp[:])
                    if e2 < e:
                        nc.vector.tensor_tensor(tmp[:], s2, se,
                                                op=alu.is_equal)
                        nc.vector.tensor_add(re, re, tmp[:])
            w_hi = rsb.tile([1, E * B], f32)
            nc.vector.tensor_scalar(w_hi[:], rank[:], float(k),
                                    1.0 / float(k),
                                    op0=alu.is_lt, op1=alu.mult)
            w_lo = rsb.tile([1, E * B], f32)
            nc.vector.tensor_scalar(w_lo[:], rank[:], float(k_lo),
                                    1.0 / float(k_lo),
                                    op0=alu.is_lt, op1=alu.mult)
            diff = rsb.tile([1, E * B], f32)
            nc.vector.tensor_sub(diff[:], w_hi[:], w_lo[:])

            # ====== router part B: softmax branch -> per-core gate ======
            ltp = mps.tile([B, NMOD], f32, tag="ps", name="ltp")
            nc.tensor.transpose(ltp[:], lr_sb[:], ident[0:NMOD, 0:NMOD])
            lt = rsb.tile([B, NMOD], f32)
            nc.vector.tensor_copy(lt[:], ltp[:])
            mx = rsb.tile([B, 1], f32)
            nc.vector.tensor_reduce(out=mx[:], in_=lt[:], op=alu.max,
                                    axis=mybir.AxisListType.X)
            mxn = rsb.tile([B, 1], f32)
            nc.vector.tensor_scalar_mul(mxn[:], mx[:], -1.0)
            ex = rsb.tile([B, NMOD], f32)
            nc.scalar.activation(ex[:], lt[:], act_fn.Exp, bias=mxn[:])
            sm = rsb.tile([B, 1], f32)
            nc.vector.tensor_reduce(out=sm[:], in_=ex[:], op=alu.add,
                                    axis=mybir.AxisListType.X)
            rs = rsb.tile([B, 1], f32)
            nc.vector.reciprocal(rs[:], sm[:])
            p0 = rsb.tile([B, 1], f32)
            nc.vector.tensor_mul(
                p0[:], ex[:, module_idx:module_idx + 1], rs[:])
            hi = rsb.tile([B, 1], f32)
            nc.vector.tensor_single_scalar(hi[:], p0[:], 0.5, alu.is_gt)
            hp = mps.tile([1, B], f32, tag="ps", name="hp")
            nc.tensor.transpose(hp[:], hi[:], ident[0:B, 0:B])
            hi_row = rsb.tile([1, B], f32)
            nc.vector.tensor_copy(hi_row[:], hp[:])
            gate = rsb.tile([1, E * B], f32)
            for e in range(E):
                nc.vector.tensor_mul(gate[:, e * B:(e + 1) * B],
                                     diff[:, e * B:(e + 1) * B],
                                     hi_row[:])
            nc.vector.tensor_add(gate[:], gate[:], w_lo[:])
            gateb = rsb.tile([ER, E * B], f32)
            nc.gpsimd.partition_broadcast(gateb[:], gate[:])
            g32m = rsb.tile([ER, E * B], f32)
            nc.vector.tensor_tensor(g32m[:], gateb[:], msel_sb[:],
                                    op=alu.mult)
            nc.vector.tensor_reduce(out=gate32[:], in_=g32m[:],
                                    op=alu.add,
                                    axis=mybir.AxisListType.X)

            # gate-scale B in place, one slice per output chunk
            bg_tiles = []
            for c in range(NCH):
                nc.vector.tensor_scalar_mul(b_all[:, c, :],
                                            b_all[:, c, :],
                                            gate32[:, 0:1])
                bg_tiles.append(b_all[:, c, :])

            # ====== phase A: chunk 0 k-outer while x streams in =========
            x_tiles = []
            wt0 = []
            psA = [mps.tile([128, 512], f32, tag="ps", name=f"psA_{mt}")
                   for mt in range(PH_A)]
            ps_t0 = mps.tile([ER, 512], f32, tag="ps", name="pst_0")
            tT = tpool.tile([ER, L], bf16)
            for kt in range(KT):
                xs = xsb_pool.tile([128, L], bf16, tag="x",
                                   name=f"x_{kt}")
                nc.sync.dma_start(xs[:], xT[kt * 128:(kt + 1) * 128, :])
                x_tiles.append(xs)
                wt = wpool.tile([128, 512], bf16, tag="w",
                                name=f"w_0_{kt}")
                nc.sync.dma_start(wt[:], Wt[0, kt])
                wt0.append(wt)
                st, sp = (kt == 0), (kt == KT - 1)
                nc.tensor.matmul(ps_t0[:], a_sb[:, kt, :], xs[:, 0:512],
                                 start=st, stop=sp)
                for mt in range(PH_A):
                    nc.tensor.matmul(psA[mt][:],
                                     xs[:, mt * 128:(mt + 1) * 128],
                                     wt[:], start=st, stop=sp)
            nc.vector.tensor_copy(tT[:, 0:512], ps_t0[:])
            for mt in range(PH_A):
                ev = epool.tile([128, 512], f32, tag="ev",
                                name=f"evA_{mt}")
                if has_bias:
                    nc.vector.tensor_add(ev[:], psA[mt][:],
                                         bias_all[0][:])
                elif mt % 2 == 0:
                    nc.vector.tensor_copy(ev[:], psA[mt][:])
                else:
                    nc.scalar.activation(ev[:], psA[mt][:], act_fn.Copy)
                nc.sync.dma_start(
                    out[mt * 128:(mt + 1) * 128, 0:512], ev[:])

            # W-tile ring: prefetch chunk c+1's tiles while chunk c runs.
            w_next = {}

            def prefetch_w(c, kts):
                if c >= NCH:
                    return
                row = w_next.setdefault(c, [None] * KT)
                for kt in kts:
                    if kt >= KT or row[kt] is not None:
                        continue
                    wt = wpool.tile([128, 512], bf16, tag="w",
                                    name=f"w_{c}_{kt}")
                    nc.sync.dma_start(wt[:], Wt[c, kt])
                    row[kt] = wt

            # ====== phase B: lora-t mg1..3 + rest of chunk 0 ===========
            for mg in range(1, MG):
                prefetch_w(1, range((mg - 1) * 3, mg * 3))
                ps_t = mps.tile([ER, 512], f32, tag="ps", name=f"pst_{mg}")
                for kt in range(KT):
                    nc.tensor.matmul(
                        ps_t[:], a_sb[:, kt, :],
                        x_tiles[kt][:, mg * 512:(mg + 1) * 512],
                        start=(kt == 0), stop=(kt == KT - 1))
                nc.vector.tensor_copy(tT[:, mg * 512:(mg + 1) * 512],
                                      ps_t[:])
            for mt in range(PH_A, MT):
                j = mt - PH_A
                prefetch_w(1, range(9 + j * 3, 12 + j * 3))
                ps = mps.tile([128, 512], f32, tag="ps", name=f"ps_0_{mt}")
                for kt in range(KT):
                    nc.tensor.matmul(
                        ps[:], x_tiles[kt][:, mt * 128:(mt + 1) * 128],
                        wt0[kt][:], start=(kt == 0), stop=(kt == KT - 1))
                ev = epool.tile([128, 512], f32, tag="ev",
                                name=f"ev_0_{mt}")
                if has_bias:
                    nc.vector.tensor_add(ev[:], ps[:], bias_all[0][:])
                elif mt % 2 == 0:
                    nc.vector.tensor_copy(ev[:], ps[:])
                else:
                    nc.scalar.activation(ev[:], ps[:], act_fn.Copy)
                nc.sync.dma_start(
                    out[mt * 128:(mt + 1) * 128, 0:512], ev[:])

            # ====== phase C: chunks 1..7 (lora MM first, then base) =====
            # chunk 2's mt loop also carries the deferred chunk-0 lora
            # (SWDGE accumulate), one m-tile per iteration.
            def emit_chunk(c):
                prefetch_w(c, range(KT))
                wt_c = w_next[c]
                for mt in range(MT):
                    prefetch_w(c + 1, range(mt * 2, mt * 2 + 2))
                    ps = mps.tile([128, 512], f32, tag="ps",
                                  name=f"ps_{c}_{mt}")
                    nc.tensor.matmul(ps[:], tT[:, mt * 128:(mt + 1) * 128],
                                     bg_tiles[c],
                                     start=True, stop=False)
                    for kt in range(KT):
                        nc.tensor.matmul(
                            ps[:], x_tiles[kt][:, mt * 128:(mt + 1) * 128],
                            wt_c[kt][:], start=False, stop=(kt == KT - 1))
                    ev = epool.tile([128, 512], f32, tag="ev",
                                    name=f"ev_{c}_{mt}")
                    if has_bias:
                        nc.vector.tensor_add(ev[:], ps[:], bias_all[c][:])
                    elif mt % 2 == 0:
                        nc.vector.tensor_copy(ev[:], ps[:])
                    else:
                        nc.scalar.activation(ev[:], ps[:], act_fn.Copy)
                    nc.sync.dma_start(
                        out[mt * 128:(mt + 1) * 128,
                            c * 512:(c + 1) * 512],
                        ev[:])
                    if c == 2:
                        lp = mps.tile([128, 512], f32, tag="ps",
                                      name=f"lp_{mt}")
                        nc.tensor.matmul(
                            lp[:], tT[:, mt * 128:(mt + 1) * 128],
                            bg_tiles[0], start=True, stop=True)
                        lev = epool.tile([128, 512], f32, tag="ev",
                                         name=f"lev_{mt}")
                        nc.vector.tensor_copy(lev[:], lp[:])
                        nc.gpsimd.dma_start(
                            out[mt * 128:(mt + 1) * 128, 0:512], lev[:],
                            accum_op=alu.add)

            for c in range(1, NCH):
                emit_chunk(c)

    nc.compile()
    return nc


def kernel(**inputs) -> np.ndarray:
    _install_profile_hook()
    import ml_dtypes
    bf = ml_dtypes.bfloat16

    x = np.asarray(inputs["x"], dtype=np.float32)
    expert_scores = np.asarray(inputs["expert_scores"], dtype=np.float32)
    W_base = np.asarray(inputs["W_base"], dtype=np.float32)
    b_base = np.asarray(inputs["b_base"], dtype=np.float32)
    gating_W = np.asarray(inputs["gating_W"], dtype=np.float32)
    W_r = np.asarray(inputs["W_r"], dtype=np.float32)
    lora_A = np.asarray(inputs["lora_A"], dtype=np.float32)
    lora_B = np.asarray(inputs["lora_B"], dtype=np.float32)
    module_idx = int(np.asarray(inputs["module_idx"]))
    k = int(np.asarray(inputs["k"]))

    has_bias = bool(np.any(b_base != 0.0))
    key = (k, module_idx, has_bias)
    if key not in _PROGRAM_CACHE:
        _PROGRAM_CACHE[key] = _build_program(k, module_idx, has_bias)
    nc = _PROGRAM_CACHE[key]

    # --- host-side layout prep (transposes/fold/bf16 rounding) ----------
    x_bf = x.astype(bf)                                  # [B, L, D]
    Wt_np = np.ascontiguousarray(
        W_base.T.reshape(KT, 128, NCH, 512).transpose(2, 0, 1, 3)
    ).astype(bf)                                         # [NCH,KT,128,512]
    C = W_r @ gating_W                                   # [NMOD, D] fp32
    ctT_np = np.ascontiguousarray(C.T).astype(bf)        # [D, NMOD]
    A_np = np.ascontiguousarray(
        lora_A.reshape(ER, D).T).astype(bf)              # [D, ER]
    B_np = np.ascontiguousarray(
        lora_B.transpose(0, 2, 1).reshape(ER, D)).astype(bf)  # [ER, D]
    scores_f_np = np.ascontiguousarray(
        expert_scores.T.reshape(1, E * B))               # [1, E*B]
    b_row_np = b_base.reshape(1, D)
    pooledT_np = np.ascontiguousarray(x[:, -1, :].T).astype(bf)  # [D, B]

    in_maps = []
    for c in range(N_CORES):
        msel_np = np.zeros((ER, E, B), dtype=np.float32)
        for p in range(ER):
            msel_np[p, p // R, c] = 1.0
        msel_np = msel_np.reshape(ER, E * B)
        in_maps.append({
            "xT": np.ascontiguousarray(x_bf[c].T),
            "Wt": Wt_np,
            "ctT": ctT_np,
            "pooledT": pooledT_np,
            "scores_f": scores_f_np,
            "A_rhs": A_np,
            "B_cat": B_np,
            "b_row": b_row_np,
            "msel": msel_np,
        })

    from concourse.bass_utils import run_bass_kernel_spmd
    res = run_bass_kernel_spmd(nc, in_maps, core_ids=list(range(N_CORES)))
    return np.stack([res.results[c]["out"] for c in range(N_CORES)], axis=0)


if __name__ == "__main__":
    rng = np.random.default_rng(0)
    demo = {
        "x": (rng.standard_normal((B, L, D)) * 0.02).astype(np.float32),
        "expert_scores": rng.random((B, E), dtype=np.float32),
        "W_base": (rng.standard_normal((D, D)) * 0.02).astype(np.float32),
        "b_base": np.zeros(D, np.float32),
        "gating_W": (rng.standard_normal((D, D)) * 0.02).astype(np.float32),
        "W_r": (rng.standard_normal((NMOD, D)) * 0.02).astype(np.float32),
        "lora_A": (rng.standard_normal((E, R, D)) * 0.02).astype(np.float32),
        "lora_B": (rng.standard_normal((E, D, R)) * 0.02).astype(np.float32),
        "module_idx": 0,
        "k": 2,
    }
    y = kernel(**demo)
    print("out", y.shape, y.dtype, float(np.abs(y).max()))


# revision 14
# speedup vs baseline: 1.3721x; 1.0005x over previous
"""DynaLoRALinear Trainium2 kernel (v3).

Data-parallel over batch B across 8 NeuronCores (one sample per core).
Per core:
  - router: logits = pooled @ C.T with C = W_r @ gating_W folded on the
    host (weight-only reassociation), so every core computes the full
    [NMOD, B] logits locally -- no collective at all.
  - gate weights from expert_scores ranks + module_prob>0.5 branch select.
  - base:   out = x_b @ W_base.T + b_base
  - lora:   tT = A_cat @ x_b.T, then out += tT.T @ (B_cat * gate)

All matmul operands are bf16. x_b^T is SBUF-resident (16 MB, 32 separate
k-tiles so compute can pace the incoming DMA stream), W_base^T streams
once through a ring pool (pre-tiled in DRAM, contiguous 128 KB tiles).
Phase A runs chunk 0 k-outer across 8 PSUM banks so the PE stays busy
while x streams in; chunk 0's LoRA term is applied later via an SWDGE
read-modify-write pass once the gate is known.
"""

import sys
import types

import numpy as np

B, L, D, E, R, NMOD = 8, 2048, 4096, 4, 8, 7
N_CORES = 8
ER = E * R          # 32
KT = D // 128       # 32 k-tiles
MT = L // 128       # 16 m-tiles
NCH = D // 512      # 8 output-column chunks
MG = L // 512       # 4 m-groups for the lora-t pass
PH_A = 7            # chunk-0 m-tiles processed k-outer during x stream


def _install_profile_hook():
    """Make bass_utils' trace path importable (no-op if already present)."""
    try:
        import antenv.axon_hooks  # noqa: F401
        return
    except ImportError:
        pass
    try:
        import antenv
    except ImportError:
        return
    mod = types.ModuleType("antenv.axon_hooks")
    mod._hook = None
    mod.set_axon_ntff_profile_hook = lambda h: setattr(mod, "_hook", h)
    mod.get_axon_ntff_profile_hook = lambda: mod._hook
    sys.modules["antenv.axon_hooks"] = mod
    antenv.axon_hooks = mod
    try:
        from trn_agent_boot.trn_boot import _ntff_profile_via_ctypes
        hook = _ntff_profile_via_ctypes("/opt/axon/libaxon_pjrt.so")
        if hook is not None:
            mod.set_axon_ntff_profile_hook(hook)
    except Exception:
        pass


_PROGRAM_CACHE = {}


def _build_program(k: int, module_idx: int, has_bias: bool):
    import concourse.mybir as mybir
    import concourse.tile as tile
    from concourse import bacc
    from concourse.masks import make_identity

    f32 = mybir.dt.float32
    bf16 = mybir.dt.bfloat16
    alu = mybir.AluOpType
    act_fn = mybir.ActivationFunctionType

    k_lo = max(1, k // 2)
    w_bufs = 36 if has_bias else 52

    nc = bacc.Bacc("TRN2", target_bir_lowering=False, debug=False,
                   num_devices=N_CORES)

    # --- DRAM I/O -------------------------------------------------------
    xT = nc.dram_tensor("xT", [D, L], bf16, kind="ExternalInput")
    Wt = nc.dram_tensor("Wt", [NCH, KT, 128, 512], bf16,
                        kind="ExternalInput")
    ctT = nc.dram_tensor("ctT", [D, NMOD], bf16, kind="ExternalInput")
    pooledT = nc.dram_tensor("pooledT", [D, B], bf16, kind="ExternalInput")
    scores_f = nc.dram_tensor("scores_f", [1, E * B], f32,
                              kind="ExternalInput")
    A_rhs = nc.dram_tensor("A_rhs", [D, ER], bf16, kind="ExternalInput")
    B_cat = nc.dram_tensor("B_cat", [ER, D], bf16, kind="ExternalInput")
    b_row = nc.dram_tensor("b_row", [1, D], f32, kind="ExternalInput")
    msel = nc.dram_tensor("msel", [ER, E * B], f32, kind="ExternalInput")
    out = nc.dram_tensor("out", [L, D], f32, kind="ExternalOutput")

    with tile.TileContext(nc) as tc:
        with (
            tc.tile_pool(name="const", bufs=1) as const_pool,
            tc.tile_pool(name="gatep", bufs=1) as gate_pool,
            tc.tile_pool(name="rsb", bufs=1) as rsb,
            tc.tile_pool(name="xsb", bufs=KT) as xsb_pool,
            tc.tile_pool(name="wpool", bufs=w_bufs) as wpool,
            tc.tile_pool(name="apool", bufs=1) as apool,
            tc.tile_pool(name="tpool", bufs=1) as tpool,
            tc.tile_pool(name="ballp", bufs=1) as ball_pool,
            tc.tile_pool(name="epool", bufs=4) as epool,
            tc.tile_pool(name="biasp",
                         bufs=(NCH if has_bias else 1)) as biasp,
            tc.tile_pool(name="mps", bufs=8, space="PSUM") as mps,
        ):
            ident = const_pool.tile([128, 128], f32)
            make_identity(nc, ident)
            gate32 = gate_pool.tile([ER, 1], f32)

            # HAM warm-up: ~24 dense matmuls on the identity tile bring
            # the PE clock gate to 8/8 while the first input DMAs stream.
            idb = const_pool.tile([128, 128], bf16)
            nc.vector.tensor_copy(idb[:], ident[:])
            warm = mps.tile([128, 128], f32, tag="ps", name="warm")
            for i in range(64):
                nc.tensor.matmul(warm[:], idb[:], idb[:],
                                 start=(i == 0), stop=(i == 63))

            bias_all = []
            if has_bias:
                for hh in range(NCH):
                    bias_bc = biasp.tile([128, 512], f32, tag="biasbc",
                                         name=f"biasbc_{hh}")
                    nc.sync.dma_start(
                        bias_bc[0:1, :],
                        b_row[:, hh * 512:(hh + 1) * 512])
                    nc.gpsimd.partition_broadcast(bias_bc[:],
                                                  bias_bc[0:1, :])
                    bias_all.append(bias_bc)

            # ====== small input DMAs ===================================
            ct_sb = rsb.tile([128, KT, NMOD], bf16)
            nc.sync.dma_start(
                ct_sb[:], ctT[:].rearrange("(a p) m -> p a m", p=128))
            pt_sb = rsb.tile([128, KT, B], bf16)
            nc.sync.dma_start(
                pt_sb[:], pooledT[:].rearrange("(a p) m -> p a m", p=128))
            msel_sb = rsb.tile([ER, E * B], f32)
            nc.sync.dma_start(msel_sb[:], msel[:])
            sc = rsb.tile([1, E * B], f32)
            nc.sync.dma_start(sc[:], scores_f[:])
            a_sb = apool.tile([128, KT, ER], bf16)
            nc.sync.dma_start(
                a_sb[:], A_rhs[:].rearrange("(a p) m -> p a m", p=128))

            # ====== router: logits = pooled @ C.T (local, no collective)
            ps_r = mps.tile([NMOD, B], f32, tag="ps", name="ps_r")
            for kt in range(KT):
                nc.tensor.matmul(ps_r[:], ct_sb[:, kt, :], pt_sb[:, kt, :],
                                 start=(kt == 0), stop=(kt == KT - 1))
            lr_sb = rsb.tile([NMOD, B], f32)
            nc.vector.tensor_copy(lr_sb[:], ps_r[:])

            # collective-independent: expert ranks from scores
            rank = rsb.tile([1, E * B], f32)
            nc.vector.memset(rank[:], 0.0)
            tmp = rsb.tile([1, B], f32)
            for e in range(E):
                re = rank[:, e * B:(e + 1) * B]
                se = sc[:, e * B:(e + 1) * B]
                for e2 in range(E):
                    if e2 == e:
                        continue
                    s2 = sc[:, e2 * B:(e2 + 1) * B]
                    nc.vector.tensor_tensor(tmp[:], s2, se, op=alu.is_gt)
                    nc.vector.tensor_add(re, re, tmp[:])
                    if e2 < e:
                        nc.vector.tensor_tensor(tmp[:], s2, se,
                                                op=alu.is_equal)
                        nc.vector.tensor_add(re, re, tmp[:])
            w_hi = rsb.tile([1, E * B], f32)
            nc.vector.tensor_scalar(w_hi[:], rank[:], float(k),
                                    1.0 / float(k),
                                    op0=alu.is_lt, op1=alu.mult)
            w_lo = rsb.tile([1, E * B], f32)
            nc.vector.tensor_scalar(w_lo[:], rank[:], float(k_lo),
                                    1.0 / float(k_lo),
                                    op0=alu.is_lt, op1=alu.mult)
            diff = rsb.tile([1, E * B], f32)
            nc.vector.tensor_sub(diff[:], w_hi[:], w_lo[:])

            # ====== router part B: softmax branch -> per-core gate ======
            ltp = mps.tile([B, NMOD], f32, tag="ps", name="ltp")
            nc.tensor.transpose(ltp[:], lr_sb[:], ident[0:NMOD, 0:NMOD])
            lt = rsb.tile([B, NMOD], f32)
            nc.vector.tensor_copy(lt[:], ltp[:])
            mx = rsb.tile([B, 1], f32)
            nc.vector.tensor_reduce(out=mx[:], in_=lt[:], op=alu.max,
                                    axis=mybir.AxisListType.X)
            mxn = rsb.tile([B, 1], f32)
            nc.vector.tensor_scalar_mul(mxn[:], mx[:], -1.0)
            ex = rsb.tile([B, NMOD], f32)
            nc.scalar.activation(ex[:], lt[:], act_fn.Exp, bias=mxn[:])
            sm = rsb.tile([B, 1], f32)
            nc.vector.tensor_reduce(out=sm[:], in_=ex[:], op=alu.add,
                                    axis=mybir.AxisListType.X)
            rs = rsb.tile([B, 1], f32)
            nc.vector.reciprocal(rs[:], sm[:])
            p0 = rsb.tile([B, 1], f32)
            nc.vector.tensor_mul(
                p0[:], ex[:, module_idx:module_idx + 1], rs[:])
            hi = rsb.tile([B, 1], f32)
            nc.vector.tensor_single_scalar(hi[:], p0[:], 0.5, alu.is_gt)
            hp = mps.tile([1, B], f32, tag="ps", name="hp")
            nc.tensor.transpose(hp[:], hi[:], ident[0:B, 0:B])
            hi_row = rsb.tile([1, B], f32)
            nc.vector.tensor_copy(hi_row[:], hp[:])
            gate = rsb.tile([1, E * B], f32)
            for e in range(E):
                nc.vector.tensor_mul(gate[:, e * B:(e + 1) * B],
                                     diff[:, e * B:(e + 1) * B],
                                     hi_row[:])
            nc.vector.tensor_add(gate[:], gate[:], w_lo[:])
            gateb = rsb.tile([ER, E * B], f32)
            nc.gpsimd.partition_broadcast(gateb[:], gate[:])
            g32m = rsb.tile([ER, E * B], f32)
            nc.vector.tensor_tensor(g32m[:], gateb[:], msel_sb[:],
                                    op=alu.mult)
            nc.vector.tensor_reduce(out=gate32[:], in_=g32m[:],
                                    op=alu.add,
                                    axis=mybir.AxisListType.X)

            # ====== phase A: chunk 0 k-outer while x streams in =========
            x_tiles = []
            wt0 = []
            psA = [mps.tile([128, 512], f32, tag="ps", name=f"psA_{mt}")
                   for mt in range(PH_A)]
            ps_t0 = mps.tile([ER, 512], f32, tag="ps", name="pst_0")
            tT = tpool.tile([ER, L], bf16)
            for kt in range(KT):
                xs = xsb_pool.tile([128, L], bf16, tag="x",
                                   name=f"x_{kt}")
                nc.sync.dma_start(xs[:], xT[kt * 128:(kt + 1) * 128, :])
                x_tiles.append(xs)
                wt = wpool.tile([128, 512], bf16, tag="w",
                                name=f"w_0_{kt}")
                nc.sync.dma_start(wt[:], Wt[0, kt])
                wt0.append(wt)
                st, sp = (kt == 0), (kt == KT - 1)
                nc.tensor.matmul(ps_t0[:], a_sb[:, kt, :], xs[:, 0:512],
                                 start=st, stop=sp)
                for mt in range(PH_A):
                    nc.tensor.matmul(psA[mt][:],
                                     xs[:, mt * 128:(mt + 1) * 128],
                                     wt[:], start=st, stop=sp)
            b_all = ball_pool.tile([ER, NCH, 512], bf16)
            nc.sync.dma_start(
                b_all[:], B_cat[:].rearrange("p (c n) -> p c n", c=NCH))
            nc.vector.tensor_copy(tT[:, 0:512], ps_t0[:])
            for mt in range(PH_A):
                ev = epool.tile([128, 512], f32, tag="ev",
                                name=f"evA_{mt}")
                if has_bias:
                    nc.vector.tensor_add(ev[:], psA[mt][:],
                                         bias_all[0][:])
                elif mt % 2 == 0:
                    nc.vector.tensor_copy(ev[:], psA[mt][:])
                else:
                    nc.scalar.activation(ev[:], psA[mt][:], act_fn.Copy)
                nc.sync.dma_start(
                    out[mt * 128:(mt + 1) * 128, 0:512], ev[:])

            # gate-scale B in place, one slice per output chunk
            bg_tiles = []
            for c in range(NCH):
                nc.vector.tensor_scalar_mul(b_all[:, c, :],
                                            b_all[:, c, :],
                                            gate32[:, 0:1])
                bg_tiles.append(b_all[:, c, :])

            # W-tile ring: prefetch chunk c+1's tiles while chunk c runs.
            w_next = {}

            def prefetch_w(c, kts):
                if c >= NCH:
                    return
                row = w_next.setdefault(c, [None] * KT)
                for kt in kts:
                    if kt >= KT or row[kt] is not None:
                        continue
                    wt = wpool.tile([128, 512], bf16, tag="w",
                                    name=f"w_{c}_{kt}")
                    nc.sync.dma_start(wt[:], Wt[c, kt])
                    row[kt] = wt

            # ====== phase B: lora-t mg1..3 + rest of chunk 0 ===========
            for mg in range(1, MG):
                prefetch_w(1, range((mg - 1) * 3, mg * 3))
                ps_t = mps.tile([ER, 512], f32, tag="ps", name=f"pst_{mg}")
                for kt in range(KT):
                    nc.tensor.matmul(
                        ps_t[:], a_sb[:, kt, :],
                        x_tiles[kt][:, mg * 512:(mg + 1) * 512],
                        start=(kt == 0), stop=(kt == KT - 1))
                nc.vector.tensor_copy(tT[:, mg * 512:(mg + 1) * 512],
                                      ps_t[:])
            for mt in range(PH_A, MT):
                j = mt - PH_A
                prefetch_w(1, range(9 + j * 3, 12 + j * 3))
                ps = mps.tile([128, 512], f32, tag="ps", name=f"ps_0_{mt}")
                for kt in range(KT):
                    nc.tensor.matmul(
                        ps[:], x_tiles[kt][:, mt * 128:(mt + 1) * 128],
                        wt0[kt][:], start=(kt == 0), stop=(kt == KT - 1))
                ev = epool.tile([128, 512], f32, tag="ev",
                                name=f"ev_0_{mt}")
                if has_bias:
                    nc.vector.tensor_add(ev[:], ps[:], bias_all[0][:])
                elif mt % 2 == 0:
                    nc.vector.tensor_copy(ev[:], ps[:])
                else:
                    nc.scalar.activation(ev[:], ps[:], act_fn.Copy)
                nc.sync.dma_start(
                    out[mt * 128:(mt + 1) * 128, 0:512], ev[:])

            # ====== phase C: chunks 1..7 (lora MM first, then base) =====
            # chunk 2's mt loop also carries the deferred chunk-0 lora
            # (SWDGE accumulate), one m-tile per iteration.
            def emit_chunk(c):
                prefetch_w(c, range(KT))
                wt_c = w_next[c]
                for mt in range(MT):
                    prefetch_w(c + 1, range(mt * 2, mt * 2 + 2))
                    ps = mps.tile([128, 512], f32, tag="ps",
                                  name=f"ps_{c}_{mt}")
                    nc.tensor.matmul(ps[:], tT[:, mt * 128:(mt + 1) * 128],
                                     bg_tiles[c],
                                     start=True, stop=False)
                    for kt in range(KT):
                        nc.tensor.matmul(
                            ps[:], x_tiles[kt][:, mt * 128:(mt + 1) * 128],
                            wt_c[kt][:], start=False, stop=(kt == KT - 1))
                    ev = epool.tile([128, 512], f32, tag="ev",
                                    name=f"ev_{c}_{mt}")
                    if has_bias:
                        nc.vector.tensor_add(ev[:], ps[:], bias_all[c][:])
                    elif mt % 2 == 0:
                        nc.vector.tensor_copy(ev[:], ps[:])
                    else:
                        nc.scalar.activation(ev[:], ps[:], act_fn.Copy)
                    nc.sync.dma_start(
                        out[mt * 128:(mt + 1) * 128,
                            c * 512:(c + 1) * 512],
                        ev[:])
                    if c == 2:
                        lp = mps.tile([128, 512], f32, tag="ps",
                                      name=f"lp_{mt}")
                        nc.tensor.matmul(
                            lp[:], tT[:, mt * 128:(mt + 1) * 128],
                            bg_tiles[0], start=True, stop=True)
                        lev = epool.tile([128, 512], f32, tag="ev",
                                         name=f"lev_{mt}")
                        nc.vector.tensor_copy(lev[:], lp[:])
                        nc.gpsimd.dma_start(
                            out[mt * 128:(mt + 1) * 128, 0:512], lev[:],
                            accum_op=alu.add)

            for c in range(1, NCH):
                emit_chunk(c)

    nc.compile()
    return nc


def kernel(**inputs) -> np.ndarray:
    _install_profile_hook()
    import ml_dtypes
    bf = ml_dtypes.bfloat16

    x = np.asarray(inputs["x"], dtype=np.float32)
    expert_scores = np.asarray(inputs["expert_scores"], dtype=np.float32)
    W_base = np.asarray(inputs["W_base"], dtype=np.float32)
    b_base = np.asarray(inputs["b_base"], dtype=np.float32)
    gating_W = np.asarray(inputs["gating_W"], dtype=np.float32)
    W_r = np.asarray(inputs["W_r"], dtype=np.float32)
    lora_A = np.asarray(inputs["lora_A"], dtype=np.float32)
    lora_B = np.asarray(inputs["lora_B"], dtype=np.float32)
    module_idx = int(np.asarray(inputs["module_idx"]))
    k = int(np.asarray(inputs["k"]))

    has_bias = bool(np.any(b_base != 0.0))
    key = (k, module_idx, has_bias)
    if key not in _PROGRAM_CACHE:
        _PROGRAM_CACHE[key] = _build_program(k, module_idx, has_bias)
    nc = _PROGRAM_CACHE[key]

    # --- host-side layout prep (transposes/fold/bf16 rounding) ----------
    x_bf = x.astype(bf)                                  # [B, L, D]
    Wt_np = np.ascontiguousarray(
        W_base.T.reshape(KT, 128, NCH, 512).transpose(2, 0, 1, 3)
    ).astype(bf)                                         # [NCH,KT,128,512]
    C = W_r @ gating_W                                   # [NMOD, D] fp32
    ctT_np = np.ascontiguousarray(C.T).astype(bf)        # [D, NMOD]
    A_np = np.ascontiguousarray(
        lora_A.reshape(ER, D).T).astype(bf)              # [D, ER]
    B_np = np.ascontiguousarray(
        lora_B.transpose(0, 2, 1).reshape(ER, D)).astype(bf)  # [ER, D]
    scores_f_np = np.ascontiguousarray(
        expert_scores.T.reshape(1, E * B))               # [1, E*B]
    b_row_np = b_base.reshape(1, D)
    pooledT_np = np.ascontiguousarray(x[:, -1, :].T).astype(bf)  # [D, B]

    in_maps = []
    for c in range(N_CORES):
        msel_np = np.zeros((ER, E, B), dtype=np.float32)
        for p in range(ER):
            msel_np[p, p // R, c] = 1.0
        msel_np = msel_np.reshape(ER, E * B)
        in_maps.append({
            "xT": np.ascontiguousarray(x_bf[c].T),
            "Wt": Wt_np,
            "ctT": ctT_np,
            "pooledT": pooledT_np,
            "scores_f": scores_f_np,
            "A_rhs": A_np,
            "B_cat": B_np,
            "b_row": b_row_np,
            "msel": msel_np,
        })

    from concourse.bass_utils import run_bass_kernel_spmd
    res = run_bass_kernel_spmd(nc, in_maps, core_ids=list(range(N_CORES)))
    return np.stack([res.results[c]["out"] for c in range(N_CORES)], axis=0)


if __name__ == "__main__":
    rng = np.random.default_rng(0)
    demo = {
        "x": (rng.standard_normal((B, L, D)) * 0.02).astype(np.float32),
        "expert_scores": rng.random((B, E), dtype=np.float32),
        "W_base": (rng.standard_normal((D, D)) * 0.02).astype(np.float32),
        "b_base": np.zeros(D, np.float32),
        "gating_W": (rng.standard_normal((D, D)) * 0.02).astype(np.float32),
        "W_r": (rng.standard_normal((NMOD, D)) * 0.02).astype(np.float32),
        "lora_A": (rng.standard_normal((E, R, D)) * 0.02).astype(np.float32),
        "lora_B": (rng.standard_normal((E, D, R)) * 0.02).astype(np.float32),
        "module_idx": 0,
        "k": 2,
    }
    y = kernel(**demo)
    print("out", y.shape, y.dtype, float(np.abs(y).max()))
